# revision 20
# baseline (speedup 1.0000x reference)
"""DMPNN (NNConv/edge-network message passing) Trainium2 kernel, 8-core SPMD.

Sharding: edges are assigned to cores by dst-node range (512 nodes/core), so
scatter-mean partial sums are core-local; per layer one AllGather of the
pre-BN activations crosses the cores (BN stats + h update are then computed
redundantly on every core).

Key idea vs the per-edge-matvec formulation: never materialize the per-edge
[H,H] weight.  With W_e = sum_k ev[e,k]*W2[k] + B2,

  msg[e,o] = sum_k ev[e,k] * U[e,(k,o)] + (hs @ B2)[e,o]
  U[e,(k,o)] = (hs @ W2[k])[e,o]

so per 128-edge tile ONE stationary (hsT, [h,e]) feeds 33 full-width matmuls
against a big fixed moving operand w2hT [128h, 129*128], and the ev scaling +
k-sum happen on the eviction path as per-partition-scalar fused ops
(scalar_tensor_tensor) split across Vector / Scalar(cast-assist) / GpSimd.
The scatter-mean is an on-chip one-hot matmul accumulating straight into a
PSUM agg tile (no DRAM staging / indirect scatter).
"""

import numpy as np
import ml_dtypes

import concourse.bass as bass
import concourse.tile as tile
import concourse.mybir as mybir
from concourse import bacc
from concourse.bass import IndirectOffsetOnAxis
from concourse.bass_utils import run_bass_kernel_spmd

BF16 = ml_dtypes.bfloat16

N, E, F_NODE, F_EDGE, H, L, G = 4096, 12288, 64, 16, 128, 4, 256
NC = 8
NS = N // NC          # nodes per core
NBLK = NS // 128      # u-blocks per core (4)
P = 128
BN_EPS = 1e-5
AXF = mybir.ActivationFunctionType
ALU = mybir.AluOpType

KB = 129              # 128 ev-k values + b2 column block
# eviction engine split (banks of 4 k each; 32 banks of real k).
# GpSimd cannot touch PSUM and has no scalar ops; its banks are scale-cast
# to SBUF by Scalar (per-k activation scale), then wide-added by GpSimd.
NV_A = 14             # banks 0..: Scalar plain-cast -> Vector STT (sbuf)
NP_A = 9              # next banks: Scalar scale-cast x4 -> GpSimd wide add
                      # rest: Vector STT directly from PSUM


# ----------------------------------------------------------------------------
# Host preprocessing
# ----------------------------------------------------------------------------

def _preprocess(edge_index, edge_attr):
    src = np.asarray(edge_index[0], dtype=np.int64)
    dst = np.asarray(edge_index[1], dtype=np.int64)
    edge_attr = np.asarray(edge_attr, dtype=np.float32)
    deg = np.bincount(dst, minlength=N).astype(np.float32)
    inv_deg = np.where(deg > 0, 1.0 / np.maximum(deg, 1.0), 0.0).astype(np.float32)

    core_of = dst // NS
    packed = []
    EP = 0
    for c in range(NC):
        idx = np.nonzero(core_of == c)[0]
        idx = idx[np.argsort(dst[idx], kind="stable")]
        d = dst[idx]
        # pack per-dst runs into 128-edge tiles; a run never crosses a tile,
        # and a tile never touches more than 2 u-blocks (forced break).
        slots = []
        fill = 0
        blocks = set()
        i = 0
        while i < len(idx):
            j = i
            while j < len(idx) and d[j] == d[i]:
                j += 1
            k = j - i
            blk = (int(d[i]) - c * NS) // P
            nb = blocks | {blk}
            if fill + k > P or (len(nb) > 2 and fill > 0):
                slots.extend([-1] * (P - fill))
                fill = 0
                blocks = set()
            slots.extend(idx[i:j].tolist())
            fill = (fill + k) % P
            blocks = set() if fill == 0 else (blocks | {blk})
            i = j
        if fill:
            slots.extend([-1] * (P - fill))
        packed.append(np.array(slots, dtype=np.int64))
        EP = max(EP, len(slots))
    EP = ((EP + P - 1) // P) * P
    T = EP // P

    cores = []
    for c in range(NC):
        slots = np.concatenate(
            [packed[c], -np.ones(EP - len(packed[c]), dtype=np.int64)]
        )
        valid = slots >= 0
        sl = np.maximum(slots, 0)
        e_src = np.where(valid, src[sl], 0).astype(np.int32)
        e_dst = np.where(valid, dst[sl] - c * NS, -1)
        ea = np.where(valid[:, None], edge_attr[sl], 0.0).astype(np.float32)
        # per tile: NBLK one-hot scatter stationaries (untouched blocks zero),
        # so the device program is identical on every core.
        oh = np.zeros((T, NBLK, P, P), np.float32)
        for t in range(T):
            ds = e_dst[t * P:(t + 1) * P]
            for e in range(P):
                if ds[e] < 0:
                    continue
                oh[t, int(ds[e]) // P, e, int(ds[e]) % P] = 1.0
        eaT = np.concatenate([ea.T, np.ones((1, EP), np.float32)], 0)  # [17,EP]
        invd = inv_deg[c * NS:(c + 1) * NS].reshape(NBLK, P).T.copy()  # [128,NBLK]
        cores.append(dict(eaT=eaT, srcg=e_src, oh=oh, invd=invd))
    return cores, EP, T


# ----------------------------------------------------------------------------
# Device program
# ----------------------------------------------------------------------------

def _build(EP, T):
    f32 = mybir.dt.float32
    bf16 = mybir.dt.bfloat16
    i32 = mybir.dt.int32
    nc = bacc.Bacc("TRN2", target_bir_lowering=False, debug=False, num_devices=NC)

    def din(name, shape, dt=bf16):
        return nc.dram_tensor(name, shape, dt, kind="ExternalInput")

    ea_d = din("ea", [17, EP])
    e1w_d = din("e1w", [L, 17, H])
    w2ht_d = din("w2ht", [L, H, KB * P])     # [h, k*128+o]; k=128 slot is b2
    rw_d = din("rw", [L, H, H])
    bng_d = din("bng", [L, H, 1], f32)
    bnb_d = din("bnb", [L, H, 1], f32)
    xa_d = din("xa", [65, N])
    xs_d = din("xs", [65, NS])
    nw_d = din("nw", [65, H])
    srcg_d = din("srcg", [P, T], i32)
    oh_d = din("oh", [P, T * NBLK * P])      # one-hot scatter stationaries
    invd_d = din("invd", [P, NBLK], f32)
    pmat_d = din("pmat", [P, 32 * G])
    hw1_d = din("hw1", [H, H])
    hb1_d = din("hb1", [H, 1], f32)
    hw2_d = din("hw2", [H, 1])
    hb2_d = din("hb2", [1, 1], f32)
    idf_d = din("idf", [P, P], f32)
    idb_d = din("idb", [P, P])
    y_d = nc.dram_tensor("y", [1, G], f32, kind="ExternalOutput")

    groups = [list(range(NC))]

    with tile.TileContext(nc) as tc:
        with tc.tile_pool(name="const", bufs=1) as const, \
             tc.tile_pool(name="persist", bufs=1) as persist, \
             tc.tile_pool(name="w2pool", bufs=2) as w2pool, \
             tc.tile_pool(name="ubpool", bufs=6) as ubpool, \
             tc.tile_pool(name="spool", bufs=3) as spool, \
             tc.tile_pool(name="mpool", bufs=3) as mpool, \
             tc.tile_pool(name="stat", bufs=2) as statp, \
             tc.tile_pool(name="psu", bufs=3, space="PSUM") as ps_u, \
             tc.tile_pool(name="psm", bufs=3, space="PSUM") as ps_m, \
             tc.tile_pool(name="psa", bufs=1, space="PSUM") as ps_agg, \
             tc.tile_pool(name="psr", bufs=1, space="PSUM") as ps_root, \
             tc.tile_pool(name="dramp", bufs=2, space="DRAM") as dramp:

            # ---- persistent constants ----
            ea_sb = const.tile([17, EP], bf16)
            nc.sync.dma_start(ea_sb[:], ea_d[:])
            nw_sb = const.tile([65, H], bf16)
            nc.sync.dma_start(nw_sb[:], nw_d[:])
            idf_sb = const.tile([P, P], f32)
            nc.sync.dma_start(idf_sb[:], idf_d[:])
            idb_sb = const.tile([P, P], bf16)
            nc.sync.dma_start(idb_sb[:], idb_d[:])
            srcg_sb = const.tile([P, T], i32)
            nc.sync.dma_start(srcg_sb[:], srcg_d[:])
            oh_sb = const.tile([P, T * NBLK * P], bf16)
            nc.sync.dma_start(oh_sb[:], oh_d[:])
            xs_sb = const.tile([65, NS], bf16)
            nc.sync.dma_start(xs_sb[:], xs_d[:])
            invd_sb = const.tile([P, NBLK], f32)
            nc.sync.dma_start(invd_sb[:], invd_d[:])
            e1w_sb = []
            rw_sb = []
            bng_sb = []
            bnb_sb = []
            for l in range(L):
                e1w_l = const.tile([17, H], bf16, name=f"e1w_{l}")
                nc.sync.dma_start(e1w_l[:], e1w_d[l])
                e1w_sb.append(e1w_l)
                rw_l = const.tile([H, H], bf16, name=f"rw_{l}")
                nc.sync.dma_start(rw_l[:], rw_d[l])
                rw_sb.append(rw_l)
                bng_l = const.tile([H, 1], f32, name=f"bng_{l}")
                nc.sync.dma_start(bng_l[:], bng_d[l])
                bng_sb.append(bng_l)
                bnb_l = const.tile([H, 1], f32, name=f"bnb_{l}")
                nc.sync.dma_start(bnb_l[:], bnb_d[l])
                bnb_sb.append(bnb_l)
            eps_sb = const.tile([H, 1], f32)
            nc.vector.memset(eps_sb[:], BN_EPS)

            hT = persist.tile([H, NS], f32)        # own slice, [h, n] fp32
            h_sb = persist.tile([P, 32 * H], bf16)  # all rows: tile j = rows
            h_dram = dramp.tile([N, H], bf16, bufs=1, name="h_dram")

            # ---- node encoder ----
            with tc.tile_pool(name="encp", bufs=3) as encp:
                xa_sb = encp.tile([65, N], bf16, bufs=1)
                nc.sync.dma_start(xa_sb[:], xa_d[:])
                for i in range(N // P):
                    ps = ps_m.tile([P, H], f32, name="enc_ps", tag="m")
                    nc.tensor.matmul(ps[:], xa_sb[:, i * P:(i + 1) * P], nw_sb[:],
                                     start=True, stop=True)
                    eng = nc.vector if i % 2 == 0 else nc.scalar
                    if i % 2 == 0:
                        eng.tensor_copy(h_sb[:, i * H:(i + 1) * H], ps[:])
                    else:
                        eng.copy(h_sb[:, i * H:(i + 1) * H], ps[:])
                    nc.sync.dma_start(h_dram[i * P:(i + 1) * P, :],
                                      h_sb[:, i * H:(i + 1) * H])
                # own slice, transposed fp32 (per-core xs input keeps the
                # program uniform across cores)
                for j in range(NS // P):
                    ps = ps_m.tile([P, H], f32, name="enc_ps2", tag="m")
                    nc.tensor.matmul(ps[:], xs_sb[:, j * P:(j + 1) * P], nw_sb[:],
                                     start=True, stop=True)
                    tmp = encp.tile([P, H], f32, name="enc_tmp")
                    nc.vector.tensor_copy(tmp[:], ps[:])
                    ps2 = ps_m.tile([P, P], f32, name="enc_ps3", tag="m")
                    nc.tensor.transpose(ps2[:], tmp[:], idf_sb[:])
                    nc.scalar.copy(hT[:, j * P:(j + 1) * P], ps2[:])

            # ---- layers ----
            for l in range(L):
                w2_sb = w2pool.tile([H, KB * P], bf16, name="w2")
                nc.sync.dma_start(w2_sb[:], w2ht_d[l])

                # ev for all tiles: [128e, 128k] f32 per tile
                ev_sb = spool.tile([P, T * P], f32, name="ev", bufs=2)
                for t in range(T):
                    pse = ps_m.tile([P, P], f32, name="ev_ps", tag="m")
                    nc.tensor.matmul(pse[:], ea_sb[:, t * P:(t + 1) * P],
                                     e1w_sb[l][:], start=True, stop=True)
                    nc.scalar.activation(ev_sb[:, t * P:(t + 1) * P], pse[:],
                                         AXF.Relu)

                # root term [128o, NS]
                hTb = spool.tile([H, NS], bf16, name="hTb")
                nc.vector.tensor_copy(hTb[:], hT[:])
                root_ps = ps_root.tile([P, NS], f32, name="root_ps", tag="r")
                nc.tensor.matmul(root_ps[:], rw_sb[l][:], hTb[:],
                                 start=True, stop=True)
                root_sb = spool.tile([H, NS], f32, name="root_sb", bufs=2)
                nc.vector.tensor_copy(root_sb[:], root_ps[:])

                # agg [u, o] accumulated in PSUM across all tiles
                agg_ps = ps_agg.tile([P, NBLK * P], f32, name="agg_ps", tag="a")
                nc.vector.memset(agg_ps[:], 0.0)

                # gather+transpose pipeline
                hs_tiles = [None] * T
                hsT_tiles = [None] * T

                def issue_gather(t):
                    hs = mpool.tile([P, H], bf16, name="hs")
                    nc.gpsimd.indirect_dma_start(
                        out=hs[:], out_offset=None, in_=h_dram[:],
                        in_offset=IndirectOffsetOnAxis(
                            ap=srcg_sb[:, t:t + 1], axis=0),
                    )
                    hs_tiles[t] = hs

                def make_hsT(t):
                    pst = ps_m.tile([P, P], bf16, name="hsT_ps", tag="m")
                    nc.tensor.transpose(pst[:], hs_tiles[t][:], idb_sb[:])
                    hsT = mpool.tile([P, P], bf16, name="hsT")
                    nc.scalar.copy(hsT[:], pst[:])
                    hsT_tiles[t] = hsT

                issue_gather(0)
                issue_gather(1)

                for t in range(T):
                    if t + 2 < T:
                        issue_gather(t + 2)
                    make_hsT(t)
                    hsT = hsT_tiles[t]

                    msgV = spool.tile([P, P], f32, name="msgV")
                    msgP4 = spool.tile([P, 512], f32, name="msgP4")

                    for b in range(32):
                        ups = ps_u.tile([P, 512], f32, name="u_ps", tag="u")
                        nc.tensor.matmul(ups[:], hsT[:],
                                         w2_sb[:, b * 512:(b + 1) * 512],
                                         start=True, stop=True)
                        if b < NV_A:
                            # Scalar casts the bank; Vector does fused
                            # scale-accumulate per k slice
                            ub = ubpool.tile([P, 512], bf16, name="ub")
                            nc.scalar.copy(ub[:], ups[:])
                            src = ub
                        elif b < NV_A + NP_A:
                            # Scalar scale-casts each k slice; GpSimd wide-adds
                            tmp4 = ubpool.tile([P, 512], bf16, name="ub")
                            for j in range(4):
                                k = b * 4 + j
                                sc = ev_sb[:, t * P + k:t * P + k + 1]
                                nc.scalar.activation(
                                    tmp4[:, j * P:(j + 1) * P],
                                    ups[:, j * P:(j + 1) * P],
                                    AXF.Copy, scale=sc)
                            if b == NV_A:
                                nc.gpsimd.tensor_copy(msgP4[:], tmp4[:])
                            else:
                                nc.gpsimd.tensor_tensor(
                                    out=msgP4[:], in0=tmp4[:], in1=msgP4[:],
                                    op=ALU.add)
                            continue
                        else:
                            src = ups
                        for j in range(4):
                            k = b * 4 + j
                            sc = ev_sb[:, t * P + k:t * P + k + 1]
                            if b == 0 and j == 0:
                                nc.vector.tensor_scalar(
                                    out=msgV[:], in0=src[:, j * P:(j + 1) * P],
                                    scalar1=sc, op0=ALU.mult,
                                    scalar2=0.0, op1=ALU.bypass)
                            else:
                                nc.vector.scalar_tensor_tensor(
                                    out=msgV[:], in0=src[:, j * P:(j + 1) * P],
                                    scalar=sc, in1=msgV[:],
                                    op0=ALU.mult, op1=ALU.add)
                    # b2 term
                    bps = ps_u.tile([P, 512], f32, name="u_ps", tag="u")
                    nc.tensor.matmul(bps[:, 0:P], hsT[:],
                                     w2_sb[:, 128 * P:129 * P],
                                     start=True, stop=True)
                    nc.vector.tensor_tensor(out=msgV[:], in0=bps[:, 0:P],
                                            in1=msgV[:], op=ALU.add)
                    # fold msgP4's 4 sub-slices + merge, bf16 for the scatter
                    nc.gpsimd.tensor_tensor(
                        out=msgP4[:, 0:P], in0=msgP4[:, 2 * P:3 * P],
                        in1=msgP4[:, 0:P], op=ALU.add)
                    nc.gpsimd.tensor_tensor(
                        out=msgP4[:, P:2 * P], in0=msgP4[:, 3 * P:4 * P],
                        in1=msgP4[:, P:2 * P], op=ALU.add)
                    nc.gpsimd.tensor_tensor(
                        out=msgP4[:, 0:P], in0=msgP4[:, P:2 * P],
                        in1=msgP4[:, 0:P], op=ALU.add)
                    msg = spool.tile([P, P], f32, name="msg")
                    nc.vector.tensor_tensor(out=msg[:], in0=msgV[:],
                                            in1=msgP4[:, 0:P], op=ALU.add)
                    msgb = spool.tile([P, P], bf16, name="msgb")
                    nc.scalar.copy(msgb[:], msg[:])

                    # scatter: one one-hot matmul per u-block (zeros for
                    # untouched blocks keep the program core-uniform)
                    for b in range(NBLK):
                        nc.tensor.matmul(
                            agg_ps[:, b * P:(b + 1) * P],
                            oh_sb[:, (t * NBLK + b) * P:(t * NBLK + b + 1) * P],
                            msgb[:], start=False, stop=False,
                            skip_group_check=True)

                # outT = aggT*invdeg + root  (per block: evict, transpose, add)
                outT = spool.tile([H, NS], f32, name="outT")
                for b in range(NBLK):
                    aggb = mpool.tile([P, P], bf16, name="aggb")
                    nc.scalar.activation(aggb[:], agg_ps[:, b * P:(b + 1) * P],
                                         AXF.Copy, scale=invd_sb[:, b:b + 1])
                    psq = ps_m.tile([P, P], bf16, name="aggT_ps", tag="m")
                    nc.tensor.transpose(psq[:], aggb[:], idb_sb[:])
                    nc.vector.tensor_tensor(
                        out=outT[:, b * P:(b + 1) * P], in0=psq[:],
                        in1=root_sb[:, b * P:(b + 1) * P], op=ALU.add)

                # ship pre-BN slice, gather all
                outTb = spool.tile([H, NS], bf16, name="outTb")
                nc.vector.tensor_copy(outTb[:], outT[:])
                outTb_dr = dramp.tile([H, NS], bf16, name="outTb_dr")
                nc.sync.dma_start(outTb_dr[:], outTb[:])
                outT_full = dramp.tile([NC * H, NS], bf16, name="outT_full",
                                       addr_space="Shared")
                nc.gpsimd.collective_compute(
                    "AllGather", ALU.bypass, replica_groups=groups,
                    ins=[outTb_dr.opt()], outs=[outT_full.opt()])
                of_sb = spool.tile([H, N], bf16, name="of_sb", bufs=1)
                for c in range(NC):
                    nc.sync.dma_start(of_sb[:, c * NS:(c + 1) * NS],
                                      outT_full[c * H:(c + 1) * H, :])

                # BN stats over all N (redundant on every core)
                stats = statp.tile([H, 2], f32, name="stats")
                nc.vector.tensor_reduce(stats[:, 0:1], of_sb[:],
                                        axis=mybir.AxisListType.X, op=ALU.add)
                trash = spool.tile([H, N], bf16, name="trash", bufs=1)
                nc.scalar.activation(trash[:], of_sb[:], AXF.Square,
                                     accum_out=stats[:, 1:2])
                mu = statp.tile([H, 1], f32, name="mu")
                nc.scalar.mul(mu[:], stats[:, 0:1], 1.0 / N)
                ex2 = statp.tile([H, 1], f32, name="ex2")
                nc.scalar.mul(ex2[:], stats[:, 1:2], 1.0 / N)
                musq = statp.tile([H, 1], f32, name="musq")
                nc.vector.tensor_mul(musq[:], mu[:], mu[:])
                var = statp.tile([H, 1], f32, name="var")
                nc.vector.tensor_tensor(out=var[:], in0=ex2[:], in1=musq[:],
                                        op=ALU.subtract)
                std = statp.tile([H, 1], f32, name="std")
                nc.scalar.activation(std[:], var[:], AXF.Sqrt,
                                     bias=eps_sb[:, 0:1])
                rstd = statp.tile([H, 1], f32, name="rstd")
                nc.vector.reciprocal(rstd[:], std[:])
                scal = statp.tile([H, 1], f32, name="scal")
                nc.vector.tensor_mul(scal[:], rstd[:], bng_sb[l][:])
                mscal = statp.tile([H, 1], f32, name="mscal")
                nc.vector.tensor_mul(mscal[:], mu[:], scal[:])
                shift = statp.tile([H, 1], f32, name="shift")
                nc.vector.tensor_tensor(out=shift[:], in0=bnb_sb[l][:],
                                        in1=mscal[:], op=ALU.subtract)

                # local hT update (f32 path)
                relu_loc = spool.tile([H, NS], f32, name="relu_loc")
                nc.scalar.activation(relu_loc[:], outT[:], AXF.Relu,
                                     bias=shift[:, 0:1], scale=scal[:, 0:1])
                nc.vector.tensor_add(hT[:], hT[:], relu_loc[:])

                # full h update (bf16 path) + DMA out for next-layer gathers
                relu_full = spool.tile([H, N], bf16, name="relu_full", bufs=1)
                nc.scalar.activation(relu_full[:], of_sb[:], AXF.Relu,
                                     bias=shift[:, 0:1], scale=scal[:, 0:1])
                for j in range(N // P):
                    psr = ps_m.tile([P, P], bf16, name="hup_ps", tag="m")
                    nc.tensor.transpose(psr[:], relu_full[:, j * P:(j + 1) * P],
                                        idb_sb[:])
                    nc.vector.tensor_tensor(out=h_sb[:, j * H:(j + 1) * H],
                                            in0=psr[:],
                                            in1=h_sb[:, j * H:(j + 1) * H],
                                            op=ALU.add)
                    nc.sync.dma_start(h_dram[j * P:(j + 1) * P, :],
                                      h_sb[:, j * H:(j + 1) * H])

            # ---- head (all cores redundantly) ----
            with tc.tile_pool(name="headp", bufs=2) as headp:
                pmat_sb = headp.tile([P, 32 * G], bf16, bufs=1)
                nc.sync.dma_start(pmat_sb[:], pmat_d[:])
                hw1_sb = headp.tile([H, H], bf16, bufs=1)
                nc.sync.dma_start(hw1_sb[:], hw1_d[:])
                hb1_sb = headp.tile([H, 1], f32, bufs=1)
                nc.sync.dma_start(hb1_sb[:], hb1_d[:])
                hw2_sb = headp.tile([H, 1], bf16, bufs=1)
                nc.sync.dma_start(hw2_sb[:], hw2_d[:])
                hb2_sb = headp.tile([1, 1], f32, bufs=1)
                nc.sync.dma_start(hb2_sb[:], hb2_d[:])

                ps_pool = ps_root.tile([H, G], f32, name="pool_ps", tag="r")
                for i in range(N // P):
                    nc.tensor.matmul(ps_pool[:], h_sb[:, i * H:(i + 1) * H],
                                     pmat_sb[:, i * G:(i + 1) * G],
                                     start=(i == 0), stop=(i == N // P - 1))
                pooledT = headp.tile([H, G], bf16, name="pooledT")
                nc.vector.tensor_copy(pooledT[:], ps_pool[:])
                ps_z = ps_m.tile([H, G], f32, name="z_ps", tag="m")
                nc.tensor.matmul(ps_z[:], hw1_sb[:], pooledT[:],
                                 start=True, stop=True)
                z = headp.tile([H, G], bf16, name="z")
                nc.scalar.activation(z[:], ps_z[:], AXF.Relu, bias=hb1_sb[:, 0:1])
                ps_y = ps_m.tile([1, G], f32, name="y_ps", tag="m")
                nc.tensor.matmul(ps_y[:], hw2_sb[:], z[:], start=True, stop=True)
                ysb = headp.tile([1, G], f32, name="ysb")
                nc.vector.tensor_scalar_add(ysb[:], ps_y[:], hb2_sb[0:1, 0:1])
                nc.sync.dma_start(y_d[:], ysb[:])

    nc.compile()
    return nc


# ----------------------------------------------------------------------------
# Entry point
# ----------------------------------------------------------------------------

def kernel(**inputs):
    inp = {k: np.asarray(v) for k, v in inputs.items()}
    cores, EP, T = _preprocess(inp["edge_index"], inp["edge_attr"])

    bf = lambda a: np.ascontiguousarray(np.asarray(a, np.float32)).astype(BF16)
    f32 = lambda a: np.ascontiguousarray(np.asarray(a, np.float32))

    # shared (replicated) tensors
    e1w = np.concatenate(
        [np.asarray(inp["e1_w"], np.float32),
         np.asarray(inp["e1_b"], np.float32)[:, None, :]], axis=1)  # [L,17,128]
    # w2hT[l][h, k*128+o] = e2_w[l][k, h*128+o]; k=128 block is e2_b
    e2w = np.asarray(inp["e2_w"], np.float32).reshape(L, H, H, H)  # [l,k,h,o]
    w2ht = np.transpose(e2w, (0, 2, 1, 3)).reshape(L, H, H * H)    # [l,h,(k,o)]
    b2 = np.asarray(inp["e2_b"], np.float32).reshape(L, H, H)      # [l,h,o]
    w2ht = np.concatenate([w2ht, b2], axis=2)                      # [l,h,129*128]
    xa = np.concatenate([np.asarray(inp["x"], np.float32).T,
                         np.ones((1, N), np.float32)], 0)  # [65, N]
    nw = np.concatenate([np.asarray(inp["node_w"], np.float32),
                         np.asarray(inp["node_b"], np.float32)[None, :]], 0)

    batch = np.asarray(inp["batch"], np.int64)
    cnt = np.bincount(batch, minlength=G).astype(np.float32)
    Pm = np.zeros((N, G), np.float32)
    Pm[np.arange(N), batch] = 1.0 / np.maximum(cnt, 1.0)[batch]
    pmat = np.zeros((P, 32 * G), np.float32)
    for i in range(32):
        pmat[:, i * G:(i + 1) * G] = Pm[i * P:(i + 1) * P]

    shared = dict(
        e1w=bf(e1w), w2ht=bf(w2ht),
        rw=bf(inp["root_w"]),
        bng=f32(inp["bn_g"])[:, :, None], bnb=f32(inp["bn_b"])[:, :, None],
        xa=bf(xa), nw=bf(nw),
        pmat=bf(pmat), hw1=bf(inp["head_w1"]),
        hb1=f32(inp["head_b1"])[:, None], hw2=bf(inp["head_w2"]),
        hb2=f32(inp["head_b2"])[None, :],
        idf=np.eye(P, dtype=np.float32),
        idb=np.eye(P, dtype=np.float32).astype(BF16),
    )

    in_maps = []
    for c in range(NC):
        cd = cores[c]
        m = dict(shared)
        m["ea"] = bf(cd["eaT"])
        m["xs"] = bf(xa[:, c * NS:(c + 1) * NS])
        m["srcg"] = np.ascontiguousarray(cd["srcg"].reshape(T, P).T)
        m["oh"] = np.ascontiguousarray(
            cd["oh"].transpose(2, 0, 1, 3).reshape(P, T * NBLK * P)).astype(BF16)
        m["invd"] = f32(cd["invd"])
        in_maps.append(m)

    nc = _build(EP, T)
    import os
    trace = os.environ.get("KERNEL_TRACE", "0") == "1"
    res = run_bass_kernel_spmd(nc, in_maps, list(range(NC)), trace=trace)
    if trace and res.exec_time_ns is not None:
        print(f"HW exec time: {res.exec_time_ns} ns")
    y = np.asarray(res.results[0]["y"], np.float32).reshape(G)
    return y


# revision 23
# speedup vs baseline: 1.0442x; 1.0442x over previous
"""DMPNN (NNConv/edge-network message passing) Trainium2 kernel, 8-core SPMD.

Sharding: edges are assigned to cores by dst-node range (512 nodes/core), so
scatter-mean partial sums are core-local; per layer one AllGather of the
pre-BN activations crosses the cores (BN stats + h update are then computed
redundantly on every core).

Key idea vs the per-edge-matvec formulation: never materialize the per-edge
[H,H] weight.  With W_e = sum_k ev[e,k]*W2[k] + B2,

  msg[e,o] = sum_k ev[e,k] * U[e,(k,o)] + (hs @ B2)[e,o]
  U[e,(k,o)] = (hs @ W2[k])[e,o]

so per 128-edge tile ONE stationary (hsT, [h,e]) feeds 33 full-width matmuls
against a big fixed moving operand w2hT [128h, 129*128], and the ev scaling +
k-sum happen on the eviction path as per-partition-scalar fused ops
(scalar_tensor_tensor) split across Vector / Scalar(cast-assist) / GpSimd.
The scatter-mean is an on-chip one-hot matmul accumulating straight into a
PSUM agg tile (no DRAM staging / indirect scatter).
"""

import numpy as np
import ml_dtypes

import concourse.bass as bass
import concourse.tile as tile
import concourse.mybir as mybir
from concourse import bacc
from concourse.bass import IndirectOffsetOnAxis
from concourse.bass_utils import run_bass_kernel_spmd

BF16 = ml_dtypes.bfloat16

N, E, F_NODE, F_EDGE, H, L, G = 4096, 12288, 64, 16, 128, 4, 256
NC = 8
NS = N // NC          # nodes per core
NBLK = NS // 128      # u-blocks per core (4)
P = 128
BN_EPS = 1e-5
AXF = mybir.ActivationFunctionType
ALU = mybir.AluOpType

KB = 129              # 128 ev-k values + b2 column block
# eviction engine split (banks of 4 k each; 32 banks of real k), balanced on
# measured op costs: V STT ~403ns/slice, V wide-bcast-mult ~700ns/bank,
# A scale-cast ~330ns/slice, Pool wide-add ~856ns/bank.
NB_VS = 10            # banks: Vector fused STT directly from PSUM
NB_VW = 8             # banks: Vector wide bcast-multiply -> GpSimd wide add
                      # rest (14): Scalar scale-cast x4 -> GpSimd wide add


# ----------------------------------------------------------------------------
# Host preprocessing
# ----------------------------------------------------------------------------

def _preprocess(edge_index, edge_attr):
    src = np.asarray(edge_index[0], dtype=np.int64)
    dst = np.asarray(edge_index[1], dtype=np.int64)
    edge_attr = np.asarray(edge_attr, dtype=np.float32)
    deg = np.bincount(dst, minlength=N).astype(np.float32)
    inv_deg = np.where(deg > 0, 1.0 / np.maximum(deg, 1.0), 0.0).astype(np.float32)

    core_of = dst // NS
    packed = []
    EP = 0
    for c in range(NC):
        idx = np.nonzero(core_of == c)[0]
        idx = idx[np.argsort(dst[idx], kind="stable")]
        d = dst[idx]
        # pack per-dst runs into 128-edge tiles; a run never crosses a tile,
        # and a tile never touches more than 2 u-blocks (forced break).
        slots = []
        fill = 0
        blocks = set()
        i = 0
        while i < len(idx):
            j = i
            while j < len(idx) and d[j] == d[i]:
                j += 1
            k = j - i
            blk = (int(d[i]) - c * NS) // P
            nb = blocks | {blk}
            if fill + k > P or (len(nb) > 2 and fill > 0):
                slots.extend([-1] * (P - fill))
                fill = 0
                blocks = set()
            slots.extend(idx[i:j].tolist())
            fill = (fill + k) % P
            blocks = set() if fill == 0 else (blocks | {blk})
            i = j
        if fill:
            slots.extend([-1] * (P - fill))
        packed.append(np.array(slots, dtype=np.int64))
        EP = max(EP, len(slots))
    EP = ((EP + P - 1) // P) * P
    T = EP // P

    cores = []
    for c in range(NC):
        slots = np.concatenate(
            [packed[c], -np.ones(EP - len(packed[c]), dtype=np.int64)]
        )
        valid = slots >= 0
        sl = np.maximum(slots, 0)
        e_src = np.where(valid, src[sl], 0).astype(np.int32)
        e_dst = np.where(valid, dst[sl] - c * NS, -1)
        ea = np.where(valid[:, None], edge_attr[sl], 0.0).astype(np.float32)
        # per tile: NBLK one-hot scatter stationaries (untouched blocks zero),
        # so the device program is identical on every core.
        oh = np.zeros((T, NBLK, P, P), np.float32)
        for t in range(T):
            ds = e_dst[t * P:(t + 1) * P]
            for e in range(P):
                if ds[e] < 0:
                    continue
                oh[t, int(ds[e]) // P, e, int(ds[e]) % P] = 1.0
        eaT = np.concatenate([ea.T, np.ones((1, EP), np.float32)], 0)  # [17,EP]
        invd = inv_deg[c * NS:(c + 1) * NS].reshape(NBLK, P).T.copy()  # [128,NBLK]
        cores.append(dict(eaT=eaT, srcg=e_src, oh=oh, invd=invd))
    return cores, EP, T


# ----------------------------------------------------------------------------
# Device program
# ----------------------------------------------------------------------------

def _build(EP, T):
    f32 = mybir.dt.float32
    bf16 = mybir.dt.bfloat16
    i32 = mybir.dt.int32
    nc = bacc.Bacc("TRN2", target_bir_lowering=False, debug=False, num_devices=NC)

    def din(name, shape, dt=bf16):
        return nc.dram_tensor(name, shape, dt, kind="ExternalInput")

    ea_d = din("ea", [17, EP])
    e1w_d = din("e1w", [L, 17, H])
    w2ht_d = din("w2ht", [L, H, KB * P])     # [h, k*128+o]; k=128 slot is b2
    rw_d = din("rw", [L, H, H])
    bng_d = din("bng", [L, H, 1], f32)
    bnb_d = din("bnb", [L, H, 1], f32)
    xa_d = din("xa", [65, N])
    xs_d = din("xs", [65, NS])
    nw_d = din("nw", [65, H])
    srcg_d = din("srcg", [P, T], i32)
    oh_d = din("oh", [P, T * NBLK * P])      # one-hot scatter stationaries
    invd_d = din("invd", [P, NBLK], f32)
    pmat_d = din("pmat", [P, 32 * G])
    hw1_d = din("hw1", [H, H])
    hb1_d = din("hb1", [H, 1], f32)
    hw2_d = din("hw2", [H, 1])
    hb2_d = din("hb2", [1, 1], f32)
    idf_d = din("idf", [P, P], f32)
    idb_d = din("idb", [P, P])
    y_d = nc.dram_tensor("y", [1, G], f32, kind="ExternalOutput")

    groups = [list(range(NC))]

    with tile.TileContext(nc) as tc:
        with tc.tile_pool(name="const", bufs=1) as const, \
             tc.tile_pool(name="persist", bufs=1) as persist, \
             tc.tile_pool(name="w2pool", bufs=2) as w2pool, \
             tc.tile_pool(name="ubpool", bufs=6) as ubpool, \
             tc.tile_pool(name="spool", bufs=3) as spool, \
             tc.tile_pool(name="mpool", bufs=3) as mpool, \
             tc.tile_pool(name="stat", bufs=2) as statp, \
             tc.tile_pool(name="psu", bufs=3, space="PSUM") as ps_u, \
             tc.tile_pool(name="psm", bufs=3, space="PSUM") as ps_m, \
             tc.tile_pool(name="psa", bufs=1, space="PSUM") as ps_agg, \
             tc.tile_pool(name="psr", bufs=1, space="PSUM") as ps_root, \
             tc.tile_pool(name="dramp", bufs=2, space="DRAM") as dramp:

            # ---- persistent constants ----
            ea_sb = const.tile([17, EP], bf16)
            nc.sync.dma_start(ea_sb[:], ea_d[:])
            nw_sb = const.tile([65, H], bf16)
            nc.sync.dma_start(nw_sb[:], nw_d[:])
            idf_sb = const.tile([P, P], f32)
            nc.sync.dma_start(idf_sb[:], idf_d[:])
            idb_sb = const.tile([P, P], bf16)
            nc.sync.dma_start(idb_sb[:], idb_d[:])
            srcg_sb = const.tile([P, T], i32)
            nc.sync.dma_start(srcg_sb[:], srcg_d[:])
            oh_sb = const.tile([P, T * NBLK * P], bf16)
            nc.sync.dma_start(oh_sb[:], oh_d[:])
            xs_sb = const.tile([65, NS], bf16)
            nc.sync.dma_start(xs_sb[:], xs_d[:])
            invd_sb = const.tile([P, NBLK], f32)
            nc.sync.dma_start(invd_sb[:], invd_d[:])
            e1w_sb = []
            rw_sb = []
            bng_sb = []
            bnb_sb = []
            for l in range(L):
                e1w_l = const.tile([17, H], bf16, name=f"e1w_{l}")
                nc.sync.dma_start(e1w_l[:], e1w_d[l])
                e1w_sb.append(e1w_l)
                rw_l = const.tile([H, H], bf16, name=f"rw_{l}")
                nc.sync.dma_start(rw_l[:], rw_d[l])
                rw_sb.append(rw_l)
                bng_l = const.tile([H, 1], f32, name=f"bng_{l}")
                nc.sync.dma_start(bng_l[:], bng_d[l])
                bng_sb.append(bng_l)
                bnb_l = const.tile([H, 1], f32, name=f"bnb_{l}")
                nc.sync.dma_start(bnb_l[:], bnb_d[l])
                bnb_sb.append(bnb_l)
            eps_sb = const.tile([H, 1], f32)
            nc.vector.memset(eps_sb[:], BN_EPS)
            z512_sb = const.tile([P, 512], f32)
            nc.vector.memset(z512_sb[:], 0.0)

            hT = persist.tile([H, NS], f32)        # own slice, [h, n] fp32
            h_sb = persist.tile([P, 32 * H], bf16)  # all rows: tile j = rows
            h_dram = dramp.tile([N, H], bf16, bufs=1, name="h_dram")

            # ---- node encoder ----
            with tc.tile_pool(name="encp", bufs=3) as encp:
                xa_sb = encp.tile([65, N], bf16, bufs=1)
                nc.sync.dma_start(xa_sb[:], xa_d[:])
                for i in range(N // P):
                    ps = ps_m.tile([P, H], f32, name="enc_ps", tag="m")
                    nc.tensor.matmul(ps[:], xa_sb[:, i * P:(i + 1) * P], nw_sb[:],
                                     start=True, stop=True)
                    eng = nc.vector if i % 2 == 0 else nc.scalar
                    if i % 2 == 0:
                        eng.tensor_copy(h_sb[:, i * H:(i + 1) * H], ps[:])
                    else:
                        eng.copy(h_sb[:, i * H:(i + 1) * H], ps[:])
                    nc.sync.dma_start(h_dram[i * P:(i + 1) * P, :],
                                      h_sb[:, i * H:(i + 1) * H])
                # own slice, transposed fp32 (per-core xs input keeps the
                # program uniform across cores)
                for j in range(NS // P):
                    ps = ps_m.tile([P, H], f32, name="enc_ps2", tag="m")
                    nc.tensor.matmul(ps[:], xs_sb[:, j * P:(j + 1) * P], nw_sb[:],
                                     start=True, stop=True)
                    tmp = encp.tile([P, H], f32, name="enc_tmp")
                    nc.vector.tensor_copy(tmp[:], ps[:])
                    ps2 = ps_m.tile([P, P], f32, name="enc_ps3", tag="m")
                    nc.tensor.transpose(ps2[:], tmp[:], idf_sb[:])
                    nc.scalar.copy(hT[:, j * P:(j + 1) * P], ps2[:])

            # ---- layers ----
            for l in range(L):
                w2_sb = w2pool.tile([H, KB * P], bf16, name="w2")
                nc.sync.dma_start(w2_sb[:], w2ht_d[l])

                # ev for all tiles: [128e, 128k] f32 per tile
                ev_sb = spool.tile([P, T * P], f32, name="ev", bufs=2)
                for t in range(T):
                    pse = ps_m.tile([P, P], f32, name="ev_ps", tag="m")
                    nc.tensor.matmul(pse[:], ea_sb[:, t * P:(t + 1) * P],
                                     e1w_sb[l][:], start=True, stop=True)
                    nc.scalar.activation(ev_sb[:, t * P:(t + 1) * P], pse[:],
                                         AXF.Relu)

                # root term [128o, NS]
                hTb = spool.tile([H, NS], bf16, name="hTb")
                nc.vector.tensor_copy(hTb[:], hT[:])
                root_ps = ps_root.tile([P, NS], f32, name="root_ps", tag="r")
                nc.tensor.matmul(root_ps[:], rw_sb[l][:], hTb[:],
                                 start=True, stop=True)
                root_sb = spool.tile([H, NS], f32, name="root_sb", bufs=2)
                nc.vector.tensor_copy(root_sb[:], root_ps[:])

                # agg [u, o] accumulated in PSUM across all tiles
                agg_ps = ps_agg.tile([P, NBLK * P], f32, name="agg_ps", tag="a")
                nc.vector.memset(agg_ps[:], 0.0)

                # gather+transpose pipeline
                hs_tiles = [None] * T
                hsT_tiles = [None] * T

                def issue_gather(t):
                    hs = mpool.tile([P, H], bf16, name="hs")
                    nc.gpsimd.indirect_dma_start(
                        out=hs[:], out_offset=None, in_=h_dram[:],
                        in_offset=IndirectOffsetOnAxis(
                            ap=srcg_sb[:, t:t + 1], axis=0),
                    )
                    hs_tiles[t] = hs

                def make_hsT(t):
                    pst = ps_m.tile([P, P], bf16, name="hsT_ps", tag="m")
                    nc.tensor.transpose(pst[:], hs_tiles[t][:], idb_sb[:])
                    hsT = mpool.tile([P, P], bf16, name="hsT")
                    nc.scalar.copy(hsT[:], pst[:])
                    hsT_tiles[t] = hsT

                issue_gather(0)
                issue_gather(1)

                for t in range(T):
                    if t + 2 < T:
                        issue_gather(t + 2)
                    make_hsT(t)
                    hsT = hsT_tiles[t]

                    msgV = spool.tile([P, P], f32, name="msgV")
                    accP = spool.tile([P, 512], f32, name="accP")
                    pool_first = True

                    for b in range(32):
                        ups = ps_u.tile([P, 512], f32, name="u_ps", tag="u")
                        nc.tensor.matmul(ups[:], hsT[:],
                                         w2_sb[:, b * 512:(b + 1) * 512],
                                         start=True, stop=True)
                        if b < NB_VS:
                            # Vector fused scale-accumulate, straight from PSUM
                            for j in range(4):
                                k = b * 4 + j
                                sc = ev_sb[:, t * P + k:t * P + k + 1]
                                if b == 0 and j == 0:
                                    nc.vector.tensor_scalar(
                                        out=msgV[:], in0=ups[:, j * P:(j + 1) * P],
                                        scalar1=sc, op0=ALU.mult,
                                        scalar2=0.0, op1=ALU.bypass)
                                else:
                                    nc.vector.scalar_tensor_tensor(
                                        out=msgV[:], in0=ups[:, j * P:(j + 1) * P],
                                        scalar=sc, in1=msgV[:],
                                        op0=ALU.mult, op1=ALU.add)
                            continue
                        tmp = ubpool.tile([P, 512], bf16, name="ub")
                        if b < NB_VS + NB_VW:
                            # Vector wide multiply with stride-0 ev broadcast
                            evb = ev_sb[:, t * P + 4 * b:t * P + 4 * b + 4]
                            nc.vector.tensor_tensor(
                                out=tmp[:].rearrange("p (a c) -> p a c", a=4),
                                in0=ups[:].rearrange("p (a c) -> p a c", a=4),
                                in1=evb[:, :, None].to_broadcast([P, 4, P]),
                                op=ALU.mult)
                        else:
                            # Scalar per-k scale-cast
                            for j in range(4):
                                k = b * 4 + j
                                sc = ev_sb[:, t * P + k:t * P + k + 1]
                                nc.scalar.activation(
                                    tmp[:, j * P:(j + 1) * P],
                                    ups[:, j * P:(j + 1) * P],
                                    AXF.Copy, scale=sc)
                        if pool_first:
                            nc.gpsimd.tensor_tensor(
                                out=accP[:], in0=tmp[:], in1=z512_sb[:],
                                op=ALU.add)
                            pool_first = False
                        else:
                            nc.gpsimd.tensor_tensor(
                                out=accP[:], in0=tmp[:], in1=accP[:],
                                op=ALU.add)
                    # b2 term
                    bps = ps_u.tile([P, 512], f32, name="u_ps", tag="u")
                    nc.tensor.matmul(bps[:, 0:P], hsT[:],
                                     w2_sb[:, 128 * P:129 * P],
                                     start=True, stop=True)
                    nc.vector.tensor_tensor(out=msgV[:], in0=bps[:, 0:P],
                                            in1=msgV[:], op=ALU.add)
                    # fold accP's 4 sub-slices + merge, bf16 for the scatter
                    nc.gpsimd.tensor_tensor(
                        out=accP[:, 0:P], in0=accP[:, 2 * P:3 * P],
                        in1=accP[:, 0:P], op=ALU.add)
                    nc.gpsimd.tensor_tensor(
                        out=accP[:, P:2 * P], in0=accP[:, 3 * P:4 * P],
                        in1=accP[:, P:2 * P], op=ALU.add)
                    nc.gpsimd.tensor_tensor(
                        out=accP[:, 0:P], in0=accP[:, P:2 * P],
                        in1=accP[:, 0:P], op=ALU.add)
                    msg = spool.tile([P, P], f32, name="msg")
                    nc.vector.tensor_tensor(out=msg[:], in0=msgV[:],
                                            in1=accP[:, 0:P], op=ALU.add)
                    msgb = spool.tile([P, P], bf16, name="msgb")
                    nc.scalar.copy(msgb[:], msg[:])

                    # scatter: one one-hot matmul per u-block (zeros for
                    # untouched blocks keep the program core-uniform)
                    for b in range(NBLK):
                        nc.tensor.matmul(
                            agg_ps[:, b * P:(b + 1) * P],
                            oh_sb[:, (t * NBLK + b) * P:(t * NBLK + b + 1) * P],
                            msgb[:], start=False, stop=False,
                            skip_group_check=True)

                # outT = aggT*invdeg + root  (per block: evict, transpose, add)
                outT = spool.tile([H, NS], f32, name="outT")
                for b in range(NBLK):
                    aggb = mpool.tile([P, P], bf16, name="aggb")
                    nc.scalar.activation(aggb[:], agg_ps[:, b * P:(b + 1) * P],
                                         AXF.Copy, scale=invd_sb[:, b:b + 1])
                    psq = ps_m.tile([P, P], bf16, name="aggT_ps", tag="m")
                    nc.tensor.transpose(psq[:], aggb[:], idb_sb[:])
                    nc.vector.tensor_tensor(
                        out=outT[:, b * P:(b + 1) * P], in0=psq[:],
                        in1=root_sb[:, b * P:(b + 1) * P], op=ALU.add)

                # ship pre-BN slice, gather all
                outTb = spool.tile([H, NS], bf16, name="outTb")
                nc.vector.tensor_copy(outTb[:], outT[:])
                outTb_dr = dramp.tile([H, NS], bf16, name="outTb_dr")
                nc.sync.dma_start(outTb_dr[:], outTb[:])
                outT_full = dramp.tile([NC * H, NS], bf16, name="outT_full",
                                       addr_space="Shared")
                nc.gpsimd.collective_compute(
                    "AllGather", ALU.bypass, replica_groups=groups,
                    ins=[outTb_dr.opt()], outs=[outT_full.opt()])
                of_sb = spool.tile([H, N], bf16, name="of_sb", bufs=1)
                for c in range(NC):
                    nc.sync.dma_start(of_sb[:, c * NS:(c + 1) * NS],
                                      outT_full[c * H:(c + 1) * H, :])

                # BN stats over all N (redundant on every core)
                stats = statp.tile([H, 2], f32, name="stats")
                nc.vector.tensor_reduce(stats[:, 0:1], of_sb[:],
                                        axis=mybir.AxisListType.X, op=ALU.add)
                trash = spool.tile([H, N], bf16, name="trash", bufs=1)
                nc.scalar.activation(trash[:], of_sb[:], AXF.Square,
                                     accum_out=stats[:, 1:2])
                mu = statp.tile([H, 1], f32, name="mu")
                nc.scalar.mul(mu[:], stats[:, 0:1], 1.0 / N)
                ex2 = statp.tile([H, 1], f32, name="ex2")
                nc.scalar.mul(ex2[:], stats[:, 1:2], 1.0 / N)
                musq = statp.tile([H, 1], f32, name="musq")
                nc.vector.tensor_mul(musq[:], mu[:], mu[:])
                var = statp.tile([H, 1], f32, name="var")
                nc.vector.tensor_tensor(out=var[:], in0=ex2[:], in1=musq[:],
                                        op=ALU.subtract)
                std = statp.tile([H, 1], f32, name="std")
                nc.scalar.activation(std[:], var[:], AXF.Sqrt,
                                     bias=eps_sb[:, 0:1])
                rstd = statp.tile([H, 1], f32, name="rstd")
                nc.vector.reciprocal(rstd[:], std[:])
                scal = statp.tile([H, 1], f32, name="scal")
                nc.vector.tensor_mul(scal[:], rstd[:], bng_sb[l][:])
                mscal = statp.tile([H, 1], f32, name="mscal")
                nc.vector.tensor_mul(mscal[:], mu[:], scal[:])
                shift = statp.tile([H, 1], f32, name="shift")
                nc.vector.tensor_tensor(out=shift[:], in0=bnb_sb[l][:],
                                        in1=mscal[:], op=ALU.subtract)

                # local hT update (f32 path)
                relu_loc = spool.tile([H, NS], f32, name="relu_loc")
                nc.scalar.activation(relu_loc[:], outT[:], AXF.Relu,
                                     bias=shift[:, 0:1], scale=scal[:, 0:1])
                nc.vector.tensor_add(hT[:], hT[:], relu_loc[:])

                # full h update (bf16 path) + DMA out for next-layer gathers
                relu_full = spool.tile([H, N], bf16, name="relu_full", bufs=1)
                nc.scalar.activation(relu_full[:], of_sb[:], AXF.Relu,
                                     bias=shift[:, 0:1], scale=scal[:, 0:1])
                for j in range(N // P):
                    psr = ps_m.tile([P, P], bf16, name="hup_ps", tag="m")
                    nc.tensor.transpose(psr[:], relu_full[:, j * P:(j + 1) * P],
                                        idb_sb[:])
                    nc.vector.tensor_tensor(out=h_sb[:, j * H:(j + 1) * H],
                                            in0=psr[:],
                                            in1=h_sb[:, j * H:(j + 1) * H],
                                            op=ALU.add)
                    nc.sync.dma_start(h_dram[j * P:(j + 1) * P, :],
                                      h_sb[:, j * H:(j + 1) * H])

            # ---- head (all cores redundantly) ----
            with tc.tile_pool(name="headp", bufs=2) as headp:
                pmat_sb = headp.tile([P, 32 * G], bf16, bufs=1)
                nc.sync.dma_start(pmat_sb[:], pmat_d[:])
                hw1_sb = headp.tile([H, H], bf16, bufs=1)
                nc.sync.dma_start(hw1_sb[:], hw1_d[:])
                hb1_sb = headp.tile([H, 1], f32, bufs=1)
                nc.sync.dma_start(hb1_sb[:], hb1_d[:])
                hw2_sb = headp.tile([H, 1], bf16, bufs=1)
                nc.sync.dma_start(hw2_sb[:], hw2_d[:])
                hb2_sb = headp.tile([1, 1], f32, bufs=1)
                nc.sync.dma_start(hb2_sb[:], hb2_d[:])

                ps_pool = ps_root.tile([H, G], f32, name="pool_ps", tag="r")
                for i in range(N // P):
                    nc.tensor.matmul(ps_pool[:], h_sb[:, i * H:(i + 1) * H],
                                     pmat_sb[:, i * G:(i + 1) * G],
                                     start=(i == 0), stop=(i == N // P - 1))
                pooledT = headp.tile([H, G], bf16, name="pooledT")
                nc.vector.tensor_copy(pooledT[:], ps_pool[:])
                ps_z = ps_m.tile([H, G], f32, name="z_ps", tag="m")
                nc.tensor.matmul(ps_z[:], hw1_sb[:], pooledT[:],
                                 start=True, stop=True)
                z = headp.tile([H, G], bf16, name="z")
                nc.scalar.activation(z[:], ps_z[:], AXF.Relu, bias=hb1_sb[:, 0:1])
                ps_y = ps_m.tile([1, G], f32, name="y_ps", tag="m")
                nc.tensor.matmul(ps_y[:], hw2_sb[:], z[:], start=True, stop=True)
                ysb = headp.tile([1, G], f32, name="ysb")
                nc.vector.tensor_scalar_add(ysb[:], ps_y[:], hb2_sb[0:1, 0:1])
                nc.sync.dma_start(y_d[:], ysb[:])

    nc.compile()
    return nc


# ----------------------------------------------------------------------------
# Entry point
# ----------------------------------------------------------------------------

def kernel(**inputs):
    inp = {k: np.asarray(v) for k, v in inputs.items()}
    cores, EP, T = _preprocess(inp["edge_index"], inp["edge_attr"])

    bf = lambda a: np.ascontiguousarray(np.asarray(a, np.float32)).astype(BF16)
    f32 = lambda a: np.ascontiguousarray(np.asarray(a, np.float32))

    # shared (replicated) tensors
    e1w = np.concatenate(
        [np.asarray(inp["e1_w"], np.float32),
         np.asarray(inp["e1_b"], np.float32)[:, None, :]], axis=1)  # [L,17,128]
    # w2hT[l][h, k*128+o] = e2_w[l][k, h*128+o]; k=128 block is e2_b
    e2w = np.asarray(inp["e2_w"], np.float32).reshape(L, H, H, H)  # [l,k,h,o]
    w2ht = np.transpose(e2w, (0, 2, 1, 3)).reshape(L, H, H * H)    # [l,h,(k,o)]
    b2 = np.asarray(inp["e2_b"], np.float32).reshape(L, H, H)      # [l,h,o]
    w2ht = np.concatenate([w2ht, b2], axis=2)                      # [l,h,129*128]
    xa = np.concatenate([np.asarray(inp["x"], np.float32).T,
                         np.ones((1, N), np.float32)], 0)  # [65, N]
    nw = np.concatenate([np.asarray(inp["node_w"], np.float32),
                         np.asarray(inp["node_b"], np.float32)[None, :]], 0)

    batch = np.asarray(inp["batch"], np.int64)
    cnt = np.bincount(batch, minlength=G).astype(np.float32)
    Pm = np.zeros((N, G), np.float32)
    Pm[np.arange(N), batch] = 1.0 / np.maximum(cnt, 1.0)[batch]
    pmat = np.zeros((P, 32 * G), np.float32)
    for i in range(32):
        pmat[:, i * G:(i + 1) * G] = Pm[i * P:(i + 1) * P]

    shared = dict(
        e1w=bf(e1w), w2ht=bf(w2ht),
        rw=bf(inp["root_w"]),
        bng=f32(inp["bn_g"])[:, :, None], bnb=f32(inp["bn_b"])[:, :, None],
        xa=bf(xa), nw=bf(nw),
        pmat=bf(pmat), hw1=bf(inp["head_w1"]),
        hb1=f32(inp["head_b1"])[:, None], hw2=bf(inp["head_w2"]),
        hb2=f32(inp["head_b2"])[None, :],
        idf=np.eye(P, dtype=np.float32),
        idb=np.eye(P, dtype=np.float32).astype(BF16),
    )

    in_maps = []
    for c in range(NC):
        cd = cores[c]
        m = dict(shared)
        m["ea"] = bf(cd["eaT"])
        m["xs"] = bf(xa[:, c * NS:(c + 1) * NS])
        m["srcg"] = np.ascontiguousarray(cd["srcg"].reshape(T, P).T)
        m["oh"] = np.ascontiguousarray(
            cd["oh"].transpose(2, 0, 1, 3).reshape(P, T * NBLK * P)).astype(BF16)
        m["invd"] = f32(cd["invd"])
        in_maps.append(m)

    nc = _build(EP, T)
    import os
    trace = os.environ.get("KERNEL_TRACE", "0") == "1"
    res = run_bass_kernel_spmd(nc, in_maps, list(range(NC)), trace=trace)
    if trace and res.exec_time_ns is not None:
        print(f"HW exec time: {res.exec_time_ns} ns")
    y = np.asarray(res.results[0]["y"], np.float32).reshape(G)
    return y


# revision 26
# speedup vs baseline: 1.3587x; 1.3011x over previous
"""DMPNN (NNConv/edge-network message passing) Trainium2 kernel, 8-core SPMD.

Sharding: edges are assigned to cores by dst-node range (512 nodes/core), so
scatter-mean partial sums are core-local; per layer one AllGather of the
pre-BN activations crosses the cores (BN stats + h update are then computed
redundantly on every core).

Key idea vs the per-edge-matvec formulation: never materialize the per-edge
[H,H] weight.  With W_e = sum_k ev[e,k]*W2[k] + B2,

  msg[e,o] = sum_k ev[e,k] * U[e,(k,o)] + (hs @ B2)[e,o]
  U[e,(k,o)] = (hs @ W2[k])[e,o]

so per 128-edge tile ONE stationary (hsT, [h,e]) feeds 33 full-width matmuls
against a big fixed moving operand w2hT [128h, 129*128], and the ev scaling +
k-sum happen on the eviction path as per-partition-scalar fused ops
(scalar_tensor_tensor) split across Vector / Scalar(cast-assist) / GpSimd.
The scatter-mean is an on-chip one-hot matmul accumulating straight into a
PSUM agg tile (no DRAM staging / indirect scatter).
"""

import numpy as np
import ml_dtypes

import concourse.bass as bass
import concourse.tile as tile
import concourse.mybir as mybir
from concourse import bacc
from concourse.bass import IndirectOffsetOnAxis
from concourse.bass_utils import run_bass_kernel_spmd

BF16 = ml_dtypes.bfloat16

N, E, F_NODE, F_EDGE, H, L, G = 4096, 12288, 64, 16, 128, 4, 256
NC = 8
NS = N // NC          # nodes per core
NBLK = NS // 128      # u-blocks per core (4)
P = 128
BN_EPS = 1e-5
AXF = mybir.ActivationFunctionType
ALU = mybir.AluOpType

KB = 129              # 128 ev-k values + b2 column block
# Eviction: every bank gets a wide multiply (Vector bcast-mult ~400ns/bank,
# or Scalar per-k scale-cast ~1320ns/bank) and a wide add into an f32
# accumulator (GpSimd ~850ns, Vector ~500ns). Paths interleave across banks
# so all engines run concurrently.
MULT_A = frozenset(b for b in range(32) if b % 3 == 2)   # Scalar mults (11)
ADD_P = frozenset(b for b in range(32) if b % 2 == 0)    # GpSimd adds (16)


# ----------------------------------------------------------------------------
# Host preprocessing
# ----------------------------------------------------------------------------

def _preprocess(edge_index, edge_attr):
    src = np.asarray(edge_index[0], dtype=np.int64)
    dst = np.asarray(edge_index[1], dtype=np.int64)
    edge_attr = np.asarray(edge_attr, dtype=np.float32)
    deg = np.bincount(dst, minlength=N).astype(np.float32)
    inv_deg = np.where(deg > 0, 1.0 / np.maximum(deg, 1.0), 0.0).astype(np.float32)

    core_of = dst // NS
    packed = []
    EP = 0
    for c in range(NC):
        idx = np.nonzero(core_of == c)[0]
        idx = idx[np.argsort(dst[idx], kind="stable")]
        d = dst[idx]
        # pack per-dst runs into 128-edge tiles; a run never crosses a tile,
        # and a tile never touches more than 2 u-blocks (forced break).
        slots = []
        fill = 0
        blocks = set()
        i = 0
        while i < len(idx):
            j = i
            while j < len(idx) and d[j] == d[i]:
                j += 1
            k = j - i
            blk = (int(d[i]) - c * NS) // P
            nb = blocks | {blk}
            if fill + k > P or (len(nb) > 2 and fill > 0):
                slots.extend([-1] * (P - fill))
                fill = 0
                blocks = set()
            slots.extend(idx[i:j].tolist())
            fill = (fill + k) % P
            blocks = set() if fill == 0 else (blocks | {blk})
            i = j
        if fill:
            slots.extend([-1] * (P - fill))
        packed.append(np.array(slots, dtype=np.int64))
        EP = max(EP, len(slots))
    EP = ((EP + P - 1) // P) * P
    T = EP // P

    cores = []
    for c in range(NC):
        slots = np.concatenate(
            [packed[c], -np.ones(EP - len(packed[c]), dtype=np.int64)]
        )
        valid = slots >= 0
        sl = np.maximum(slots, 0)
        e_src = np.where(valid, src[sl], 0).astype(np.int32)
        e_dst = np.where(valid, dst[sl] - c * NS, -1)
        ea = np.where(valid[:, None], edge_attr[sl], 0.0).astype(np.float32)
        # per tile: NBLK one-hot scatter stationaries (untouched blocks zero),
        # so the device program is identical on every core.
        oh = np.zeros((T, NBLK, P, P), np.float32)
        for t in range(T):
            ds = e_dst[t * P:(t + 1) * P]
            for e in range(P):
                if ds[e] < 0:
                    continue
                oh[t, int(ds[e]) // P, e, int(ds[e]) % P] = 1.0
        eaT = np.concatenate([ea.T, np.ones((1, EP), np.float32)], 0)  # [17,EP]
        invd = inv_deg[c * NS:(c + 1) * NS].reshape(NBLK, P).T.copy()  # [128,NBLK]
        cores.append(dict(eaT=eaT, srcg=e_src, oh=oh, invd=invd))
    return cores, EP, T


# ----------------------------------------------------------------------------
# Device program
# ----------------------------------------------------------------------------

def _build(EP, T):
    f32 = mybir.dt.float32
    bf16 = mybir.dt.bfloat16
    i32 = mybir.dt.int32
    nc = bacc.Bacc("TRN2", target_bir_lowering=False, debug=False, num_devices=NC)

    def din(name, shape, dt=bf16):
        return nc.dram_tensor(name, shape, dt, kind="ExternalInput")

    ea_d = din("ea", [17, EP])
    e1w_d = din("e1w", [L, 17, H])
    w2ht_d = din("w2ht", [L, H, KB * P])     # [h, k*128+o]; k=128 slot is b2
    rw_d = din("rw", [L, H, H])
    bng_d = din("bng", [L, H, 1], f32)
    bnb_d = din("bnb", [L, H, 1], f32)
    xa_d = din("xa", [65, N])
    xs_d = din("xs", [65, NS])
    nw_d = din("nw", [65, H])
    srcg_d = din("srcg", [P, T], i32)
    oh_d = din("oh", [P, T * NBLK * P])      # one-hot scatter stationaries
    invd_d = din("invd", [P, NBLK], f32)
    pmat_d = din("pmat", [P, 32 * G])
    hw1_d = din("hw1", [H, H])
    hb1_d = din("hb1", [H, 1], f32)
    hw2_d = din("hw2", [H, 1])
    hb2_d = din("hb2", [1, 1], f32)
    idf_d = din("idf", [P, P], f32)
    idb_d = din("idb", [P, P])
    y_d = nc.dram_tensor("y", [1, G], f32, kind="ExternalOutput")

    groups = [list(range(NC))]

    with tile.TileContext(nc) as tc:
        with tc.tile_pool(name="const", bufs=1) as const, \
             tc.tile_pool(name="persist", bufs=1) as persist, \
             tc.tile_pool(name="w2pool", bufs=2) as w2pool, \
             tc.tile_pool(name="ubpool", bufs=6) as ubpool, \
             tc.tile_pool(name="spool", bufs=3) as spool, \
             tc.tile_pool(name="mpool", bufs=3) as mpool, \
             tc.tile_pool(name="stat", bufs=2) as statp, \
             tc.tile_pool(name="psu", bufs=4, space="PSUM") as ps_u, \
             tc.tile_pool(name="psm", bufs=2, space="PSUM") as ps_m, \
             tc.tile_pool(name="psa", bufs=1, space="PSUM") as ps_agg, \
             tc.tile_pool(name="psr", bufs=1, space="PSUM") as ps_root, \
             tc.tile_pool(name="dramp", bufs=2, space="DRAM") as dramp:

            # ---- persistent constants ----
            ea_sb = const.tile([17, EP], bf16)
            nc.sync.dma_start(ea_sb[:], ea_d[:])
            nw_sb = const.tile([65, H], bf16)
            nc.sync.dma_start(nw_sb[:], nw_d[:])
            idf_sb = const.tile([P, P], f32)
            nc.sync.dma_start(idf_sb[:], idf_d[:])
            idb_sb = const.tile([P, P], bf16)
            nc.sync.dma_start(idb_sb[:], idb_d[:])
            srcg_sb = const.tile([P, T], i32)
            nc.sync.dma_start(srcg_sb[:], srcg_d[:])
            oh_sb = const.tile([P, T * NBLK * P], bf16)
            nc.sync.dma_start(oh_sb[:], oh_d[:])
            xs_sb = const.tile([65, NS], bf16)
            nc.sync.dma_start(xs_sb[:], xs_d[:])
            invd_sb = const.tile([P, NBLK], f32)
            nc.sync.dma_start(invd_sb[:], invd_d[:])
            e1w_sb = []
            rw_sb = []
            bng_sb = []
            bnb_sb = []
            for l in range(L):
                e1w_l = const.tile([17, H], bf16, name=f"e1w_{l}")
                nc.sync.dma_start(e1w_l[:], e1w_d[l])
                e1w_sb.append(e1w_l)
                rw_l = const.tile([H, H], bf16, name=f"rw_{l}")
                nc.sync.dma_start(rw_l[:], rw_d[l])
                rw_sb.append(rw_l)
                bng_l = const.tile([H, 1], f32, name=f"bng_{l}")
                nc.sync.dma_start(bng_l[:], bng_d[l])
                bng_sb.append(bng_l)
                bnb_l = const.tile([H, 1], f32, name=f"bnb_{l}")
                nc.sync.dma_start(bnb_l[:], bnb_d[l])
                bnb_sb.append(bnb_l)
            eps_sb = const.tile([H, 1], f32)
            nc.vector.memset(eps_sb[:], BN_EPS)
            z512_sb = const.tile([P, 512], f32)
            nc.vector.memset(z512_sb[:], 0.0)

            hT = persist.tile([H, NS], f32)        # own slice, [h, n] fp32
            h_sb = persist.tile([P, 32 * H], bf16)  # all rows: tile j = rows
            h_dram = dramp.tile([N, H], bf16, bufs=1, name="h_dram")

            # ---- node encoder ----
            with tc.tile_pool(name="encp", bufs=3) as encp:
                xa_sb = encp.tile([65, N], bf16, bufs=1)
                nc.sync.dma_start(xa_sb[:], xa_d[:])
                for i in range(N // P):
                    ps = ps_m.tile([P, H], f32, name="enc_ps", tag="m")
                    nc.tensor.matmul(ps[:], xa_sb[:, i * P:(i + 1) * P], nw_sb[:],
                                     start=True, stop=True)
                    eng = nc.vector if i % 2 == 0 else nc.scalar
                    if i % 2 == 0:
                        eng.tensor_copy(h_sb[:, i * H:(i + 1) * H], ps[:])
                    else:
                        eng.copy(h_sb[:, i * H:(i + 1) * H], ps[:])
                    nc.sync.dma_start(h_dram[i * P:(i + 1) * P, :],
                                      h_sb[:, i * H:(i + 1) * H])
                # own slice, transposed fp32 (per-core xs input keeps the
                # program uniform across cores)
                for j in range(NS // P):
                    ps = ps_m.tile([P, H], f32, name="enc_ps2", tag="m")
                    nc.tensor.matmul(ps[:], xs_sb[:, j * P:(j + 1) * P], nw_sb[:],
                                     start=True, stop=True)
                    tmp = encp.tile([P, H], f32, name="enc_tmp")
                    nc.vector.tensor_copy(tmp[:], ps[:])
                    ps2 = ps_m.tile([P, P], f32, name="enc_ps3", tag="m")
                    nc.tensor.transpose(ps2[:], tmp[:], idf_sb[:])
                    nc.scalar.copy(hT[:, j * P:(j + 1) * P], ps2[:])

            # ---- layers ----
            for l in range(L):
                w2_sb = w2pool.tile([H, KB * P], bf16, name="w2")
                nc.sync.dma_start(w2_sb[:], w2ht_d[l])

                # ev for all tiles: [128e, 128k] f32 per tile
                ev_sb = spool.tile([P, T * P], f32, name="ev", bufs=2)
                for t in range(T):
                    pse = ps_m.tile([P, P], f32, name="ev_ps", tag="m")
                    nc.tensor.matmul(pse[:], ea_sb[:, t * P:(t + 1) * P],
                                     e1w_sb[l][:], start=True, stop=True)
                    nc.scalar.activation(ev_sb[:, t * P:(t + 1) * P], pse[:],
                                         AXF.Relu)

                # root term [128o, NS]
                hTb = spool.tile([H, NS], bf16, name="hTb")
                nc.vector.tensor_copy(hTb[:], hT[:])
                root_ps = ps_root.tile([P, NS], f32, name="root_ps", tag="r")
                nc.tensor.matmul(root_ps[:], rw_sb[l][:], hTb[:],
                                 start=True, stop=True)
                root_sb = spool.tile([H, NS], f32, name="root_sb", bufs=2)
                nc.vector.tensor_copy(root_sb[:], root_ps[:])

                # agg [u, o] accumulated in PSUM across all tiles
                agg_ps = ps_agg.tile([P, NBLK * P], f32, name="agg_ps", tag="a")
                nc.vector.memset(agg_ps[:], 0.0)

                # gather+transpose pipeline
                hs_tiles = [None] * T
                hsT_tiles = [None] * T

                def issue_gather(t):
                    hs = mpool.tile([P, H], bf16, name="hs")
                    nc.gpsimd.indirect_dma_start(
                        out=hs[:], out_offset=None, in_=h_dram[:],
                        in_offset=IndirectOffsetOnAxis(
                            ap=srcg_sb[:, t:t + 1], axis=0),
                    )
                    hs_tiles[t] = hs

                def make_hsT(t):
                    pst = ps_m.tile([P, P], bf16, name="hsT_ps", tag="m")
                    nc.tensor.transpose(pst[:], hs_tiles[t][:], idb_sb[:])
                    hsT = mpool.tile([P, P], bf16, name="hsT")
                    nc.scalar.copy(hsT[:], pst[:])
                    hsT_tiles[t] = hsT

                issue_gather(0)
                issue_gather(1)

                for t in range(T):
                    if t + 2 < T:
                        issue_gather(t + 2)
                    make_hsT(t)
                    hsT = hsT_tiles[t]

                    accV = spool.tile([P, 512], f32, name="accV")
                    accP = spool.tile([P, 512], f32, name="accP")
                    first = {id(accV): True, id(accP): True}

                    for b in range(32):
                        ups = ps_u.tile([P, 512], f32, name="u_ps", tag="u")
                        nc.tensor.matmul(ups[:], hsT[:],
                                         w2_sb[:, b * 512:(b + 1) * 512],
                                         start=True, stop=True)
                        tmp = ubpool.tile([P, 512], bf16, name="ub")
                        if b in MULT_A:
                            # Scalar per-k scale-cast
                            for j in range(4):
                                k = b * 4 + j
                                sc = ev_sb[:, t * P + k:t * P + k + 1]
                                nc.scalar.activation(
                                    tmp[:, j * P:(j + 1) * P],
                                    ups[:, j * P:(j + 1) * P],
                                    AXF.Copy, scale=sc)
                        else:
                            # Vector wide multiply with stride-0 ev broadcast
                            evb = ev_sb[:, t * P + 4 * b:t * P + 4 * b + 4]
                            nc.vector.tensor_tensor(
                                out=tmp[:].rearrange("p (a c) -> p a c", a=4),
                                in0=ups[:].rearrange("p (a c) -> p a c", a=4),
                                in1=evb[:, :, None].to_broadcast([P, 4, P]),
                                op=ALU.mult)
                        eng = nc.gpsimd if b in ADD_P else nc.vector
                        acc = accP if b in ADD_P else accV
                        if first[id(acc)]:
                            eng.tensor_tensor(out=acc[:], in0=tmp[:],
                                              in1=z512_sb[:], op=ALU.add)
                            first[id(acc)] = False
                        else:
                            eng.tensor_tensor(out=acc[:], in0=tmp[:],
                                              in1=acc[:], op=ALU.add)
                    # b2 term into accV slot 0
                    bps = ps_u.tile([P, 512], f32, name="u_ps", tag="u")
                    nc.tensor.matmul(bps[:, 0:P], hsT[:],
                                     w2_sb[:, 128 * P:129 * P],
                                     start=True, stop=True)
                    nc.vector.tensor_tensor(out=accV[:, 0:P], in0=bps[:, 0:P],
                                            in1=accV[:, 0:P], op=ALU.add)
                    # merge accumulators, fold the 4 k-slots, bf16 for scatter
                    nc.vector.tensor_tensor(out=accV[:], in0=accP[:],
                                            in1=accV[:], op=ALU.add)
                    nc.vector.tensor_tensor(
                        out=accV[:, 0:P], in0=accV[:, 2 * P:3 * P],
                        in1=accV[:, 0:P], op=ALU.add)
                    nc.gpsimd.tensor_tensor(
                        out=accV[:, P:2 * P], in0=accV[:, 3 * P:4 * P],
                        in1=accV[:, P:2 * P], op=ALU.add)
                    msg = spool.tile([P, P], f32, name="msg")
                    nc.vector.tensor_tensor(out=msg[:], in0=accV[:, 0:P],
                                            in1=accV[:, P:2 * P], op=ALU.add)
                    msgb = spool.tile([P, P], bf16, name="msgb")
                    nc.scalar.copy(msgb[:], msg[:])

                    # scatter: one one-hot matmul per u-block (zeros for
                    # untouched blocks keep the program core-uniform)
                    for b in range(NBLK):
                        nc.tensor.matmul(
                            agg_ps[:, b * P:(b + 1) * P],
                            oh_sb[:, (t * NBLK + b) * P:(t * NBLK + b + 1) * P],
                            msgb[:], start=False, stop=False,
                            skip_group_check=True)

                # outT = aggT*invdeg + root  (per block: evict, transpose, add)
                outT = spool.tile([H, NS], f32, name="outT")
                for b in range(NBLK):
                    aggb = mpool.tile([P, P], bf16, name="aggb")
                    nc.scalar.activation(aggb[:], agg_ps[:, b * P:(b + 1) * P],
                                         AXF.Copy, scale=invd_sb[:, b:b + 1])
                    psq = ps_m.tile([P, P], bf16, name="aggT_ps", tag="m")
                    nc.tensor.transpose(psq[:], aggb[:], idb_sb[:])
                    nc.vector.tensor_tensor(
                        out=outT[:, b * P:(b + 1) * P], in0=psq[:],
                        in1=root_sb[:, b * P:(b + 1) * P], op=ALU.add)

                # ship pre-BN slice, gather all
                outTb = spool.tile([H, NS], bf16, name="outTb")
                nc.vector.tensor_copy(outTb[:], outT[:])
                outTb_dr = dramp.tile([H, NS], bf16, name="outTb_dr")
                nc.sync.dma_start(outTb_dr[:], outTb[:])
                outT_full = dramp.tile([NC * H, NS], bf16, name="outT_full",
                                       addr_space="Shared")
                nc.gpsimd.collective_compute(
                    "AllGather", ALU.bypass, replica_groups=groups,
                    ins=[outTb_dr.opt()], outs=[outT_full.opt()])
                of_sb = spool.tile([H, N], bf16, name="of_sb", bufs=1)
                for c in range(NC):
                    nc.sync.dma_start(of_sb[:, c * NS:(c + 1) * NS],
                                      outT_full[c * H:(c + 1) * H, :])

                # BN stats over all N (redundant on every core)
                stats = statp.tile([H, 2], f32, name="stats")
                nc.vector.tensor_reduce(stats[:, 0:1], of_sb[:],
                                        axis=mybir.AxisListType.X, op=ALU.add)
                trash = spool.tile([H, N], bf16, name="trash", bufs=1)
                nc.scalar.activation(trash[:], of_sb[:], AXF.Square,
                                     accum_out=stats[:, 1:2])
                mu = statp.tile([H, 1], f32, name="mu")
                nc.scalar.mul(mu[:], stats[:, 0:1], 1.0 / N)
                ex2 = statp.tile([H, 1], f32, name="ex2")
                nc.scalar.mul(ex2[:], stats[:, 1:2], 1.0 / N)
                musq = statp.tile([H, 1], f32, name="musq")
                nc.vector.tensor_mul(musq[:], mu[:], mu[:])
                var = statp.tile([H, 1], f32, name="var")
                nc.vector.tensor_tensor(out=var[:], in0=ex2[:], in1=musq[:],
                                        op=ALU.subtract)
                std = statp.tile([H, 1], f32, name="std")
                nc.scalar.activation(std[:], var[:], AXF.Sqrt,
                                     bias=eps_sb[:, 0:1])
                rstd = statp.tile([H, 1], f32, name="rstd")
                nc.vector.reciprocal(rstd[:], std[:])
                scal = statp.tile([H, 1], f32, name="scal")
                nc.vector.tensor_mul(scal[:], rstd[:], bng_sb[l][:])
                mscal = statp.tile([H, 1], f32, name="mscal")
                nc.vector.tensor_mul(mscal[:], mu[:], scal[:])
                shift = statp.tile([H, 1], f32, name="shift")
                nc.vector.tensor_tensor(out=shift[:], in0=bnb_sb[l][:],
                                        in1=mscal[:], op=ALU.subtract)

                # local hT update (f32 path)
                relu_loc = spool.tile([H, NS], f32, name="relu_loc")
                nc.scalar.activation(relu_loc[:], outT[:], AXF.Relu,
                                     bias=shift[:, 0:1], scale=scal[:, 0:1])
                nc.vector.tensor_add(hT[:], hT[:], relu_loc[:])

                # full h update (bf16 path) + DMA out for next-layer gathers
                relu_full = spool.tile([H, N], bf16, name="relu_full", bufs=1)
                nc.scalar.activation(relu_full[:], of_sb[:], AXF.Relu,
                                     bias=shift[:, 0:1], scale=scal[:, 0:1])
                for j in range(N // P):
                    psr = ps_m.tile([P, P], bf16, name="hup_ps", tag="m")
                    nc.tensor.transpose(psr[:], relu_full[:, j * P:(j + 1) * P],
                                        idb_sb[:])
                    nc.vector.tensor_tensor(out=h_sb[:, j * H:(j + 1) * H],
                                            in0=psr[:],
                                            in1=h_sb[:, j * H:(j + 1) * H],
                                            op=ALU.add)
                    nc.sync.dma_start(h_dram[j * P:(j + 1) * P, :],
                                      h_sb[:, j * H:(j + 1) * H])

            # ---- head (all cores redundantly) ----
            with tc.tile_pool(name="headp", bufs=2) as headp:
                pmat_sb = headp.tile([P, 32 * G], bf16, bufs=1)
                nc.sync.dma_start(pmat_sb[:], pmat_d[:])
                hw1_sb = headp.tile([H, H], bf16, bufs=1)
                nc.sync.dma_start(hw1_sb[:], hw1_d[:])
                hb1_sb = headp.tile([H, 1], f32, bufs=1)
                nc.sync.dma_start(hb1_sb[:], hb1_d[:])
                hw2_sb = headp.tile([H, 1], bf16, bufs=1)
                nc.sync.dma_start(hw2_sb[:], hw2_d[:])
                hb2_sb = headp.tile([1, 1], f32, bufs=1)
                nc.sync.dma_start(hb2_sb[:], hb2_d[:])

                ps_pool = ps_root.tile([H, G], f32, name="pool_ps", tag="r")
                for i in range(N // P):
                    nc.tensor.matmul(ps_pool[:], h_sb[:, i * H:(i + 1) * H],
                                     pmat_sb[:, i * G:(i + 1) * G],
                                     start=(i == 0), stop=(i == N // P - 1))
                pooledT = headp.tile([H, G], bf16, name="pooledT")
                nc.vector.tensor_copy(pooledT[:], ps_pool[:])
                ps_z = ps_m.tile([H, G], f32, name="z_ps", tag="m")
                nc.tensor.matmul(ps_z[:], hw1_sb[:], pooledT[:],
                                 start=True, stop=True)
                z = headp.tile([H, G], bf16, name="z")
                nc.scalar.activation(z[:], ps_z[:], AXF.Relu, bias=hb1_sb[:, 0:1])
                ps_y = ps_m.tile([1, G], f32, name="y_ps", tag="m")
                nc.tensor.matmul(ps_y[:], hw2_sb[:], z[:], start=True, stop=True)
                ysb = headp.tile([1, G], f32, name="ysb")
                nc.vector.tensor_scalar_add(ysb[:], ps_y[:], hb2_sb[0:1, 0:1])
                nc.sync.dma_start(y_d[:], ysb[:])

    nc.compile()
    return nc


# ----------------------------------------------------------------------------
# Entry point
# ----------------------------------------------------------------------------

def kernel(**inputs):
    inp = {k: np.asarray(v) for k, v in inputs.items()}
    cores, EP, T = _preprocess(inp["edge_index"], inp["edge_attr"])

    bf = lambda a: np.ascontiguousarray(np.asarray(a, np.float32)).astype(BF16)
    f32 = lambda a: np.ascontiguousarray(np.asarray(a, np.float32))

    # shared (replicated) tensors
    e1w = np.concatenate(
        [np.asarray(inp["e1_w"], np.float32),
         np.asarray(inp["e1_b"], np.float32)[:, None, :]], axis=1)  # [L,17,128]
    # w2hT[l][h, k*128+o] = e2_w[l][k, h*128+o]; k=128 block is e2_b
    e2w = np.asarray(inp["e2_w"], np.float32).reshape(L, H, H, H)  # [l,k,h,o]
    w2ht = np.transpose(e2w, (0, 2, 1, 3)).reshape(L, H, H * H)    # [l,h,(k,o)]
    b2 = np.asarray(inp["e2_b"], np.float32).reshape(L, H, H)      # [l,h,o]
    w2ht = np.concatenate([w2ht, b2], axis=2)                      # [l,h,129*128]
    xa = np.concatenate([np.asarray(inp["x"], np.float32).T,
                         np.ones((1, N), np.float32)], 0)  # [65, N]
    nw = np.concatenate([np.asarray(inp["node_w"], np.float32),
                         np.asarray(inp["node_b"], np.float32)[None, :]], 0)

    batch = np.asarray(inp["batch"], np.int64)
    cnt = np.bincount(batch, minlength=G).astype(np.float32)
    Pm = np.zeros((N, G), np.float32)
    Pm[np.arange(N), batch] = 1.0 / np.maximum(cnt, 1.0)[batch]
    pmat = np.zeros((P, 32 * G), np.float32)
    for i in range(32):
        pmat[:, i * G:(i + 1) * G] = Pm[i * P:(i + 1) * P]

    shared = dict(
        e1w=bf(e1w), w2ht=bf(w2ht),
        rw=bf(inp["root_w"]),
        bng=f32(inp["bn_g"])[:, :, None], bnb=f32(inp["bn_b"])[:, :, None],
        xa=bf(xa), nw=bf(nw),
        pmat=bf(pmat), hw1=bf(inp["head_w1"]),
        hb1=f32(inp["head_b1"])[:, None], hw2=bf(inp["head_w2"]),
        hb2=f32(inp["head_b2"])[None, :],
        idf=np.eye(P, dtype=np.float32),
        idb=np.eye(P, dtype=np.float32).astype(BF16),
    )

    in_maps = []
    for c in range(NC):
        cd = cores[c]
        m = dict(shared)
        m["ea"] = bf(cd["eaT"])
        m["xs"] = bf(xa[:, c * NS:(c + 1) * NS])
        m["srcg"] = np.ascontiguousarray(cd["srcg"].reshape(T, P).T)
        m["oh"] = np.ascontiguousarray(
            cd["oh"].transpose(2, 0, 1, 3).reshape(P, T * NBLK * P)).astype(BF16)
        m["invd"] = f32(cd["invd"])
        in_maps.append(m)

    nc = _build(EP, T)
    import os
    trace = os.environ.get("KERNEL_TRACE", "0") == "1"
    res = run_bass_kernel_spmd(nc, in_maps, list(range(NC)), trace=trace)
    if trace and res.exec_time_ns is not None:
        print(f"HW exec time: {res.exec_time_ns} ns")
    y = np.asarray(res.results[0]["y"], np.float32).reshape(G)
    return y


# revision 29
# speedup vs baseline: 1.4235x; 1.0477x over previous
"""DMPNN (NNConv/edge-network message passing) Trainium2 kernel, 8-core SPMD.

Sharding: edges are assigned to cores by dst-node range (512 nodes/core), so
scatter-mean partial sums are core-local; per layer one AllGather of the
pre-BN activations crosses the cores (BN stats + h update are then computed
redundantly on every core).

Key idea vs the per-edge-matvec formulation: never materialize the per-edge
[H,H] weight.  With W_e = sum_k ev[e,k]*W2[k] + B2,

  msg[e,o] = sum_k ev[e,k] * U[e,(k,o)] + (hs @ B2)[e,o]
  U[e,(k,o)] = (hs @ W2[k])[e,o]

so per 128-edge tile ONE stationary (hsT, [h,e]) feeds 33 full-width matmuls
against a big fixed moving operand w2hT [128h, 129*128], and the ev scaling +
k-sum happen on the eviction path as per-partition-scalar fused ops
(scalar_tensor_tensor) split across Vector / Scalar(cast-assist) / GpSimd.
The scatter-mean is an on-chip one-hot matmul accumulating straight into a
PSUM agg tile (no DRAM staging / indirect scatter).
"""

import numpy as np
import ml_dtypes

import concourse.bass as bass
import concourse.tile as tile
import concourse.mybir as mybir
from concourse import bacc
from concourse.bass import IndirectOffsetOnAxis
from concourse.bass_utils import run_bass_kernel_spmd

BF16 = ml_dtypes.bfloat16

N, E, F_NODE, F_EDGE, H, L, G = 4096, 12288, 64, 16, 128, 4, 256
NC = 8
NS = N // NC          # nodes per core
NBLK = NS // 128      # u-blocks per core (4)
P = 128
BN_EPS = 1e-5
AXF = mybir.ActivationFunctionType
ALU = mybir.AluOpType

KB = 129              # 128 ev-k values + b2 column block
# Eviction: every bank gets a wide multiply (Vector bcast-mult ~400ns/bank,
# or Scalar per-k scale-cast ~1320ns/bank) and a wide add into an f32
# accumulator (GpSimd ~850ns, Vector ~500ns). Paths interleave across banks
# so all engines run concurrently.
MULT_A = frozenset({1, 4, 7, 10, 13, 16, 19, 22, 25, 28})               # (10)
ADD_P = frozenset({0, 2, 5, 7, 9, 11, 14, 16, 18, 21, 23, 25, 28, 30})  # (14)


# ----------------------------------------------------------------------------
# Host preprocessing
# ----------------------------------------------------------------------------

def _preprocess(edge_index, edge_attr):
    src = np.asarray(edge_index[0], dtype=np.int64)
    dst = np.asarray(edge_index[1], dtype=np.int64)
    edge_attr = np.asarray(edge_attr, dtype=np.float32)
    deg = np.bincount(dst, minlength=N).astype(np.float32)
    inv_deg = np.where(deg > 0, 1.0 / np.maximum(deg, 1.0), 0.0).astype(np.float32)

    core_of = dst // NS
    packed = []
    EP = 0
    for c in range(NC):
        idx = np.nonzero(core_of == c)[0]
        idx = idx[np.argsort(dst[idx], kind="stable")]
        d = dst[idx]
        # pack per-dst runs into 128-edge tiles; a run never crosses a tile,
        # and a tile never touches more than 2 u-blocks (forced break).
        slots = []
        fill = 0
        blocks = set()
        i = 0
        while i < len(idx):
            j = i
            while j < len(idx) and d[j] == d[i]:
                j += 1
            k = j - i
            blk = (int(d[i]) - c * NS) // P
            nb = blocks | {blk}
            if fill + k > P or (len(nb) > 2 and fill > 0):
                slots.extend([-1] * (P - fill))
                fill = 0
                blocks = set()
            slots.extend(idx[i:j].tolist())
            fill = (fill + k) % P
            blocks = set() if fill == 0 else (blocks | {blk})
            i = j
        if fill:
            slots.extend([-1] * (P - fill))
        packed.append(np.array(slots, dtype=np.int64))
        EP = max(EP, len(slots))
    EP = ((EP + P - 1) // P) * P
    T = EP // P

    cores = []
    for c in range(NC):
        slots = np.concatenate(
            [packed[c], -np.ones(EP - len(packed[c]), dtype=np.int64)]
        )
        valid = slots >= 0
        sl = np.maximum(slots, 0)
        e_src = np.where(valid, src[sl], 0).astype(np.int32)
        e_dst = np.where(valid, dst[sl] - c * NS, -1)
        ea = np.where(valid[:, None], edge_attr[sl], 0.0).astype(np.float32)
        # per tile: NBLK one-hot scatter stationaries (untouched blocks zero),
        # so the device program is identical on every core.
        oh = np.zeros((T, NBLK, P, P), np.float32)
        for t in range(T):
            ds = e_dst[t * P:(t + 1) * P]
            for e in range(P):
                if ds[e] < 0:
                    continue
                oh[t, int(ds[e]) // P, e, int(ds[e]) % P] = 1.0
        eaT = np.concatenate([ea.T, np.ones((1, EP), np.float32)], 0)  # [17,EP]
        invd = inv_deg[c * NS:(c + 1) * NS].reshape(NBLK, P).T.copy()  # [128,NBLK]
        cores.append(dict(eaT=eaT, srcg=e_src, oh=oh, invd=invd))
    return cores, EP, T


# ----------------------------------------------------------------------------
# Device program
# ----------------------------------------------------------------------------

def _build(EP, T):
    f32 = mybir.dt.float32
    bf16 = mybir.dt.bfloat16
    i32 = mybir.dt.int32
    nc = bacc.Bacc("TRN2", target_bir_lowering=False, debug=False, num_devices=NC)

    def din(name, shape, dt=bf16):
        return nc.dram_tensor(name, shape, dt, kind="ExternalInput")

    ea_d = din("ea", [17, EP])
    e1w_d = din("e1w", [L, 17, H])
    w2ht_d = din("w2ht", [L, H, KB * P])     # [h, k*128+o]; k=128 slot is b2
    rw_d = din("rw", [L, H, H])
    bng_d = din("bng", [L, H, 1], f32)
    bnb_d = din("bnb", [L, H, 1], f32)
    xa_d = din("xa", [65, N])
    xs_d = din("xs", [65, NS])
    nw_d = din("nw", [65, H])
    srcg_d = din("srcg", [P, T], i32)
    oh_d = din("oh", [P, T * NBLK * P])      # one-hot scatter stationaries
    invd_d = din("invd", [P, NBLK], f32)
    pmat_d = din("pmat", [P, 32 * G])
    hw1_d = din("hw1", [H, H])
    hb1_d = din("hb1", [H, 1], f32)
    hw2_d = din("hw2", [H, 1])
    hb2_d = din("hb2", [1, 1], f32)
    idf_d = din("idf", [P, P], f32)
    idb_d = din("idb", [P, P])
    y_d = nc.dram_tensor("y", [1, G], f32, kind="ExternalOutput")

    groups = [list(range(NC))]

    with tile.TileContext(nc) as tc:
        with tc.tile_pool(name="const", bufs=1) as const, \
             tc.tile_pool(name="persist", bufs=1) as persist, \
             tc.tile_pool(name="w2pool", bufs=2) as w2pool, \
             tc.tile_pool(name="ubpool", bufs=6) as ubpool, \
             tc.tile_pool(name="spool", bufs=3) as spool, \
             tc.tile_pool(name="mpool", bufs=3) as mpool, \
             tc.tile_pool(name="stat", bufs=2) as statp, \
             tc.tile_pool(name="psu", bufs=4, space="PSUM") as ps_u, \
             tc.tile_pool(name="psm", bufs=2, space="PSUM") as ps_m, \
             tc.tile_pool(name="psa", bufs=1, space="PSUM") as ps_agg, \
             tc.tile_pool(name="psr", bufs=1, space="PSUM") as ps_root, \
             tc.tile_pool(name="dramp", bufs=2, space="DRAM") as dramp:

            # ---- persistent constants ----
            ea_sb = const.tile([17, EP], bf16)
            nc.sync.dma_start(ea_sb[:], ea_d[:])
            nw_sb = const.tile([65, H], bf16)
            nc.sync.dma_start(nw_sb[:], nw_d[:])
            idf_sb = const.tile([P, P], f32)
            nc.sync.dma_start(idf_sb[:], idf_d[:])
            idb_sb = const.tile([P, P], bf16)
            nc.sync.dma_start(idb_sb[:], idb_d[:])
            srcg_sb = const.tile([P, T], i32)
            nc.sync.dma_start(srcg_sb[:], srcg_d[:])
            oh_sb = const.tile([P, T * NBLK * P], bf16)
            nc.sync.dma_start(oh_sb[:], oh_d[:])
            xs_sb = const.tile([65, NS], bf16)
            nc.sync.dma_start(xs_sb[:], xs_d[:])
            invd_sb = const.tile([P, NBLK], f32)
            nc.sync.dma_start(invd_sb[:], invd_d[:])
            e1w_sb = []
            rw_sb = []
            bng_sb = []
            bnb_sb = []
            for l in range(L):
                e1w_l = const.tile([17, H], bf16, name=f"e1w_{l}")
                nc.sync.dma_start(e1w_l[:], e1w_d[l])
                e1w_sb.append(e1w_l)
                rw_l = const.tile([H, H], bf16, name=f"rw_{l}")
                nc.sync.dma_start(rw_l[:], rw_d[l])
                rw_sb.append(rw_l)
                bng_l = const.tile([H, 1], f32, name=f"bng_{l}")
                nc.sync.dma_start(bng_l[:], bng_d[l])
                bng_sb.append(bng_l)
                bnb_l = const.tile([H, 1], f32, name=f"bnb_{l}")
                nc.sync.dma_start(bnb_l[:], bnb_d[l])
                bnb_sb.append(bnb_l)
            eps_sb = const.tile([H, 1], f32)
            nc.vector.memset(eps_sb[:], BN_EPS)
            z512_sb = const.tile([P, 512], f32)
            nc.vector.memset(z512_sb[:], 0.0)

            hT = persist.tile([H, NS], f32)        # own slice, [h, n] fp32
            h_sb = persist.tile([P, 32 * H], bf16)  # all rows: tile j = rows
            h_dram = dramp.tile([N, H], bf16, bufs=1, name="h_dram")

            # ---- node encoder ----
            with tc.tile_pool(name="encp", bufs=3) as encp:
                xa_sb = encp.tile([65, N], bf16, bufs=1)
                nc.sync.dma_start(xa_sb[:], xa_d[:])
                for i in range(N // P):
                    ps = ps_m.tile([P, H], f32, name="enc_ps", tag="m")
                    nc.tensor.matmul(ps[:], xa_sb[:, i * P:(i + 1) * P], nw_sb[:],
                                     start=True, stop=True)
                    eng = nc.vector if i % 2 == 0 else nc.scalar
                    if i % 2 == 0:
                        eng.tensor_copy(h_sb[:, i * H:(i + 1) * H], ps[:])
                    else:
                        eng.copy(h_sb[:, i * H:(i + 1) * H], ps[:])
                    nc.sync.dma_start(h_dram[i * P:(i + 1) * P, :],
                                      h_sb[:, i * H:(i + 1) * H])
                # own slice, transposed fp32 (per-core xs input keeps the
                # program uniform across cores)
                for j in range(NS // P):
                    ps = ps_m.tile([P, H], f32, name="enc_ps2", tag="m")
                    nc.tensor.matmul(ps[:], xs_sb[:, j * P:(j + 1) * P], nw_sb[:],
                                     start=True, stop=True)
                    tmp = encp.tile([P, H], f32, name="enc_tmp")
                    nc.vector.tensor_copy(tmp[:], ps[:])
                    ps2 = ps_m.tile([P, P], f32, name="enc_ps3", tag="m")
                    nc.tensor.transpose(ps2[:], tmp[:], idf_sb[:])
                    nc.scalar.copy(hT[:, j * P:(j + 1) * P], ps2[:])

            # ---- layers ----
            for l in range(L):
                w2_sb = w2pool.tile([H, KB * P], bf16, name="w2")
                nc.sync.dma_start(w2_sb[:], w2ht_d[l])

                # ev for all tiles: [128e, 128k] f32 per tile
                ev_sb = spool.tile([P, T * P], f32, name="ev", bufs=2)
                for t in range(T):
                    pse = ps_m.tile([P, P], f32, name="ev_ps", tag="m")
                    nc.tensor.matmul(pse[:], ea_sb[:, t * P:(t + 1) * P],
                                     e1w_sb[l][:], start=True, stop=True)
                    nc.scalar.activation(ev_sb[:, t * P:(t + 1) * P], pse[:],
                                         AXF.Relu)

                # root term [128o, NS]
                hTb = spool.tile([H, NS], bf16, name="hTb")
                nc.vector.tensor_copy(hTb[:], hT[:])
                root_ps = ps_root.tile([P, NS], f32, name="root_ps", tag="r")
                nc.tensor.matmul(root_ps[:], rw_sb[l][:], hTb[:],
                                 start=True, stop=True)
                root_sb = spool.tile([H, NS], f32, name="root_sb", bufs=2)
                nc.vector.tensor_copy(root_sb[:], root_ps[:])

                # agg [u, o] accumulated in PSUM across all tiles
                agg_ps = ps_agg.tile([P, NBLK * P], f32, name="agg_ps", tag="a")
                nc.vector.memset(agg_ps[:], 0.0)

                # gather+transpose pipeline
                hs_tiles = [None] * T
                hsT_tiles = [None] * T

                def issue_gather(t):
                    hs = mpool.tile([P, H], bf16, name="hs")
                    nc.gpsimd.indirect_dma_start(
                        out=hs[:], out_offset=None, in_=h_dram[:],
                        in_offset=IndirectOffsetOnAxis(
                            ap=srcg_sb[:, t:t + 1], axis=0),
                    )
                    hs_tiles[t] = hs

                def make_hsT(t):
                    pst = ps_m.tile([P, P], bf16, name="hsT_ps", tag="m")
                    nc.tensor.transpose(pst[:], hs_tiles[t][:], idb_sb[:])
                    hsT = mpool.tile([P, P], bf16, name="hsT")
                    nc.scalar.copy(hsT[:], pst[:])
                    hsT_tiles[t] = hsT

                issue_gather(0)
                issue_gather(1)

                for t in range(T):
                    if t + 2 < T:
                        issue_gather(t + 2)
                    make_hsT(t)
                    hsT = hsT_tiles[t]

                    # two bf16 Vector accumulators (2x DVE adds, short chains)
                    # + one f32 GpSimd accumulator
                    accV0 = spool.tile([P, 512], bf16, name="accV0")
                    accV1 = spool.tile([P, 512], bf16, name="accV1")
                    accP = spool.tile([P, 512], f32, name="accP")
                    first = {id(accV0): True, id(accV1): True, id(accP): True}
                    nv = 0

                    for b in range(32):
                        ups = ps_u.tile([P, 512], f32, name="u_ps", tag="u")
                        nc.tensor.matmul(ups[:], hsT[:],
                                         w2_sb[:, b * 512:(b + 1) * 512],
                                         start=True, stop=True)
                        tmp = ubpool.tile([P, 512], bf16, name="ub")
                        if b in MULT_A:
                            # Scalar per-k scale-cast
                            for j in range(4):
                                k = b * 4 + j
                                sc = ev_sb[:, t * P + k:t * P + k + 1]
                                nc.scalar.activation(
                                    tmp[:, j * P:(j + 1) * P],
                                    ups[:, j * P:(j + 1) * P],
                                    AXF.Copy, scale=sc)
                        else:
                            # Vector wide multiply with stride-0 ev broadcast
                            evb = ev_sb[:, t * P + 4 * b:t * P + 4 * b + 4]
                            nc.vector.tensor_tensor(
                                out=tmp[:].rearrange("p (a c) -> p a c", a=4),
                                in0=ups[:].rearrange("p (a c) -> p a c", a=4),
                                in1=evb[:, :, None].to_broadcast([P, 4, P]),
                                op=ALU.mult)
                        if b in ADD_P:
                            eng, acc = nc.gpsimd, accP
                        else:
                            eng, acc = nc.vector, (accV0 if nv % 2 == 0 else accV1)
                            nv += 1
                        if first[id(acc)]:
                            eng.tensor_copy(acc[:], tmp[:])
                            first[id(acc)] = False
                        else:
                            eng.tensor_tensor(out=acc[:], in0=tmp[:],
                                              in1=acc[:], op=ALU.add)
                    # b2 term
                    bps = ps_u.tile([P, 512], f32, name="u_ps", tag="u")
                    nc.tensor.matmul(bps[:, 0:P], hsT[:],
                                     w2_sb[:, 128 * P:129 * P],
                                     start=True, stop=True)
                    # merge: accV0+accV1 (2x), + accP, fold 4 k-slots, + b2
                    nc.vector.tensor_tensor(out=accV0[:], in0=accV1[:],
                                            in1=accV0[:], op=ALU.add)
                    accM = spool.tile([P, 512], f32, name="accM")
                    nc.vector.tensor_tensor(out=accM[:], in0=accV0[:],
                                            in1=accP[:], op=ALU.add)
                    nc.vector.tensor_tensor(
                        out=accM[:, 0:P], in0=accM[:, 2 * P:3 * P],
                        in1=accM[:, 0:P], op=ALU.add)
                    nc.gpsimd.tensor_tensor(
                        out=accM[:, P:2 * P], in0=accM[:, 3 * P:4 * P],
                        in1=accM[:, P:2 * P], op=ALU.add)
                    nc.vector.tensor_tensor(out=accM[:, 0:P], in0=accM[:, P:2 * P],
                                            in1=accM[:, 0:P], op=ALU.add)
                    msg = spool.tile([P, P], f32, name="msg")
                    nc.vector.tensor_tensor(out=msg[:], in0=bps[:, 0:P],
                                            in1=accM[:, 0:P], op=ALU.add)
                    msgb = spool.tile([P, P], bf16, name="msgb")
                    nc.scalar.copy(msgb[:], msg[:])

                    # scatter: one one-hot matmul per u-block (zeros for
                    # untouched blocks keep the program core-uniform)
                    for b in range(NBLK):
                        nc.tensor.matmul(
                            agg_ps[:, b * P:(b + 1) * P],
                            oh_sb[:, (t * NBLK + b) * P:(t * NBLK + b + 1) * P],
                            msgb[:], start=False, stop=False,
                            skip_group_check=True)

                # outT = aggT*invdeg + root  (per block: evict, transpose, add)
                outT = spool.tile([H, NS], f32, name="outT")
                for b in range(NBLK):
                    aggb = mpool.tile([P, P], bf16, name="aggb")
                    nc.scalar.activation(aggb[:], agg_ps[:, b * P:(b + 1) * P],
                                         AXF.Copy, scale=invd_sb[:, b:b + 1])
                    psq = ps_m.tile([P, P], bf16, name="aggT_ps", tag="m")
                    nc.tensor.transpose(psq[:], aggb[:], idb_sb[:])
                    nc.vector.tensor_tensor(
                        out=outT[:, b * P:(b + 1) * P], in0=psq[:],
                        in1=root_sb[:, b * P:(b + 1) * P], op=ALU.add)

                # ship pre-BN slice, gather all
                outTb = spool.tile([H, NS], bf16, name="outTb")
                nc.vector.tensor_copy(outTb[:], outT[:])
                outTb_dr = dramp.tile([H, NS], bf16, name="outTb_dr")
                nc.sync.dma_start(outTb_dr[:], outTb[:])
                outT_full = dramp.tile([NC * H, NS], bf16, name="outT_full",
                                       addr_space="Shared")
                nc.gpsimd.collective_compute(
                    "AllGather", ALU.bypass, replica_groups=groups,
                    ins=[outTb_dr.opt()], outs=[outT_full.opt()])
                of_sb = spool.tile([H, N], bf16, name="of_sb", bufs=1)
                for c in range(NC):
                    nc.sync.dma_start(of_sb[:, c * NS:(c + 1) * NS],
                                      outT_full[c * H:(c + 1) * H, :])

                # BN stats over all N (redundant on every core)
                stats = statp.tile([H, 2], f32, name="stats")
                nc.vector.tensor_reduce(stats[:, 0:1], of_sb[:],
                                        axis=mybir.AxisListType.X, op=ALU.add)
                trash = spool.tile([H, N], bf16, name="trash", bufs=1)
                nc.scalar.activation(trash[:], of_sb[:], AXF.Square,
                                     accum_out=stats[:, 1:2])
                mu = statp.tile([H, 1], f32, name="mu")
                nc.scalar.mul(mu[:], stats[:, 0:1], 1.0 / N)
                ex2 = statp.tile([H, 1], f32, name="ex2")
                nc.scalar.mul(ex2[:], stats[:, 1:2], 1.0 / N)
                musq = statp.tile([H, 1], f32, name="musq")
                nc.vector.tensor_mul(musq[:], mu[:], mu[:])
                var = statp.tile([H, 1], f32, name="var")
                nc.vector.tensor_tensor(out=var[:], in0=ex2[:], in1=musq[:],
                                        op=ALU.subtract)
                std = statp.tile([H, 1], f32, name="std")
                nc.scalar.activation(std[:], var[:], AXF.Sqrt,
                                     bias=eps_sb[:, 0:1])
                rstd = statp.tile([H, 1], f32, name="rstd")
                nc.vector.reciprocal(rstd[:], std[:])
                scal = statp.tile([H, 1], f32, name="scal")
                nc.vector.tensor_mul(scal[:], rstd[:], bng_sb[l][:])
                mscal = statp.tile([H, 1], f32, name="mscal")
                nc.vector.tensor_mul(mscal[:], mu[:], scal[:])
                shift = statp.tile([H, 1], f32, name="shift")
                nc.vector.tensor_tensor(out=shift[:], in0=bnb_sb[l][:],
                                        in1=mscal[:], op=ALU.subtract)

                # local hT update (f32 path)
                relu_loc = spool.tile([H, NS], f32, name="relu_loc")
                nc.scalar.activation(relu_loc[:], outT[:], AXF.Relu,
                                     bias=shift[:, 0:1], scale=scal[:, 0:1])
                nc.vector.tensor_add(hT[:], hT[:], relu_loc[:])

                # full h update (bf16 path) + DMA out for next-layer gathers
                relu_full = spool.tile([H, N], bf16, name="relu_full", bufs=1)
                nc.scalar.activation(relu_full[:], of_sb[:], AXF.Relu,
                                     bias=shift[:, 0:1], scale=scal[:, 0:1])
                for j in range(N // P):
                    psr = ps_m.tile([P, P], bf16, name="hup_ps", tag="m")
                    nc.tensor.transpose(psr[:], relu_full[:, j * P:(j + 1) * P],
                                        idb_sb[:])
                    nc.vector.tensor_tensor(out=h_sb[:, j * H:(j + 1) * H],
                                            in0=psr[:],
                                            in1=h_sb[:, j * H:(j + 1) * H],
                                            op=ALU.add)
                    nc.sync.dma_start(h_dram[j * P:(j + 1) * P, :],
                                      h_sb[:, j * H:(j + 1) * H])

            # ---- head (all cores redundantly) ----
            with tc.tile_pool(name="headp", bufs=2) as headp:
                pmat_sb = headp.tile([P, 32 * G], bf16, bufs=1)
                nc.sync.dma_start(pmat_sb[:], pmat_d[:])
                hw1_sb = headp.tile([H, H], bf16, bufs=1)
                nc.sync.dma_start(hw1_sb[:], hw1_d[:])
                hb1_sb = headp.tile([H, 1], f32, bufs=1)
                nc.sync.dma_start(hb1_sb[:], hb1_d[:])
                hw2_sb = headp.tile([H, 1], bf16, bufs=1)
                nc.sync.dma_start(hw2_sb[:], hw2_d[:])
                hb2_sb = headp.tile([1, 1], f32, bufs=1)
                nc.sync.dma_start(hb2_sb[:], hb2_d[:])

                ps_pool = ps_root.tile([H, G], f32, name="pool_ps", tag="r")
                for i in range(N // P):
                    nc.tensor.matmul(ps_pool[:], h_sb[:, i * H:(i + 1) * H],
                                     pmat_sb[:, i * G:(i + 1) * G],
                                     start=(i == 0), stop=(i == N // P - 1))
                pooledT = headp.tile([H, G], bf16, name="pooledT")
                nc.vector.tensor_copy(pooledT[:], ps_pool[:])
                ps_z = ps_m.tile([H, G], f32, name="z_ps", tag="m")
                nc.tensor.matmul(ps_z[:], hw1_sb[:], pooledT[:],
                                 start=True, stop=True)
                z = headp.tile([H, G], bf16, name="z")
                nc.scalar.activation(z[:], ps_z[:], AXF.Relu, bias=hb1_sb[:, 0:1])
                ps_y = ps_m.tile([1, G], f32, name="y_ps", tag="m")
                nc.tensor.matmul(ps_y[:], hw2_sb[:], z[:], start=True, stop=True)
                ysb = headp.tile([1, G], f32, name="ysb")
                nc.vector.tensor_scalar_add(ysb[:], ps_y[:], hb2_sb[0:1, 0:1])
                nc.sync.dma_start(y_d[:], ysb[:])

    nc.compile()
    return nc


# ----------------------------------------------------------------------------
# Entry point
# ----------------------------------------------------------------------------

def kernel(**inputs):
    inp = {k: np.asarray(v) for k, v in inputs.items()}
    cores, EP, T = _preprocess(inp["edge_index"], inp["edge_attr"])

    bf = lambda a: np.ascontiguousarray(np.asarray(a, np.float32)).astype(BF16)
    f32 = lambda a: np.ascontiguousarray(np.asarray(a, np.float32))

    # shared (replicated) tensors
    e1w = np.concatenate(
        [np.asarray(inp["e1_w"], np.float32),
         np.asarray(inp["e1_b"], np.float32)[:, None, :]], axis=1)  # [L,17,128]
    # w2hT[l][h, k*128+o] = e2_w[l][k, h*128+o]; k=128 block is e2_b
    e2w = np.asarray(inp["e2_w"], np.float32).reshape(L, H, H, H)  # [l,k,h,o]
    w2ht = np.transpose(e2w, (0, 2, 1, 3)).reshape(L, H, H * H)    # [l,h,(k,o)]
    b2 = np.asarray(inp["e2_b"], np.float32).reshape(L, H, H)      # [l,h,o]
    w2ht = np.concatenate([w2ht, b2], axis=2)                      # [l,h,129*128]
    xa = np.concatenate([np.asarray(inp["x"], np.float32).T,
                         np.ones((1, N), np.float32)], 0)  # [65, N]
    nw = np.concatenate([np.asarray(inp["node_w"], np.float32),
                         np.asarray(inp["node_b"], np.float32)[None, :]], 0)

    batch = np.asarray(inp["batch"], np.int64)
    cnt = np.bincount(batch, minlength=G).astype(np.float32)
    Pm = np.zeros((N, G), np.float32)
    Pm[np.arange(N), batch] = 1.0 / np.maximum(cnt, 1.0)[batch]
    pmat = np.zeros((P, 32 * G), np.float32)
    for i in range(32):
        pmat[:, i * G:(i + 1) * G] = Pm[i * P:(i + 1) * P]

    shared = dict(
        e1w=bf(e1w), w2ht=bf(w2ht),
        rw=bf(inp["root_w"]),
        bng=f32(inp["bn_g"])[:, :, None], bnb=f32(inp["bn_b"])[:, :, None],
        xa=bf(xa), nw=bf(nw),
        pmat=bf(pmat), hw1=bf(inp["head_w1"]),
        hb1=f32(inp["head_b1"])[:, None], hw2=bf(inp["head_w2"]),
        hb2=f32(inp["head_b2"])[None, :],
        idf=np.eye(P, dtype=np.float32),
        idb=np.eye(P, dtype=np.float32).astype(BF16),
    )

    in_maps = []
    for c in range(NC):
        cd = cores[c]
        m = dict(shared)
        m["ea"] = bf(cd["eaT"])
        m["xs"] = bf(xa[:, c * NS:(c + 1) * NS])
        m["srcg"] = np.ascontiguousarray(cd["srcg"].reshape(T, P).T)
        m["oh"] = np.ascontiguousarray(
            cd["oh"].transpose(2, 0, 1, 3).reshape(P, T * NBLK * P)).astype(BF16)
        m["invd"] = f32(cd["invd"])
        in_maps.append(m)

    nc = _build(EP, T)
    import os
    trace = os.environ.get("KERNEL_TRACE", "0") == "1"
    res = run_bass_kernel_spmd(nc, in_maps, list(range(NC)), trace=trace)
    if trace and res.exec_time_ns is not None:
        print(f"HW exec time: {res.exec_time_ns} ns")
    y = np.asarray(res.results[0]["y"], np.float32).reshape(G)
    return y
